# revision 1
# baseline (speedup 1.0000x reference)
"""DeepseekV2 MLA attention (T=2048, H=16) on 8 trn2 cores.

Sharding: tensor-parallel over heads (2 heads/core); the low-rank
a-projections (hidden @ w_qa / w_kva) are replicated on every core
(computed in bf16 to fit SBUF); per-core partial outputs (local heads
through o_proj) are summed on the host.

Device-side layout tricks:
- All attention operands are kept "transposed" ([feature, t]) so every
  matmul contraction lands on the partition dim with no PE transposes.
- Scores are computed as S^T[k, q] = K^T q blocks; softmax denominator
  comes from a ones-vector matmul (partition-dim reduction on the PE);
  no row-max is subtracted (scaled scores are ~N(0,1); exp is safe in
  fp32) and normalization is applied after P@V (linear).
- RMSNorm is applied post-matmul: r[t]=rsqrt(mean(x^2)+eps) is computed
  via squares + ones-matmul, and multiplied into the up-projected
  outputs (ln weights are folded into the b-projections on the host).
- Neox rope is folded into duplicated/rotated weight columns so the
  rotate-half never crosses partitions on the DVE.
- Matmuls run as fp32r (1 cyc/row at free-dim >= 256) except stage-1
  which is bf16.
"""

import numpy as np

T = 2048
HID = 2048
H = 16
NC_ = 8
HLOC = H // NC_          # 2 heads per core
QL = 1536                # q lora
KVL = 512                # kv lora
DN = 128                 # nope dim
DR = 64                  # rope dim
DQK = DN + DR            # 192
DV = 128
EPS = 1e-6
SCALE = float(DQK) ** -0.5
P = 128
CH = 512                 # stage-A t-chunk
NCH = T // CH
QC = 512                 # attention q-chunk
NQC = T // QC
NKB = T // P             # key blocks

_CACHE = {}
LAST_RESULTS = None



def _split_multi_waits(nc, mybir):
    """Walrus embeds at most one sem/event wait per TPB instruction; hoist
    extra waits onto preceding same-engine NoOps (queue FIFO keeps order)."""
    n = 0
    for f in nc.m.functions:
        for bb in f.blocks:
            new = []
            for inst in bb.instructions:
                si = getattr(inst, "sync_info", None)
                if si is not None and len(si.on_wait) > 1:
                    waits = list(si.on_wait)
                    for i, wv in enumerate(waits[:-1]):
                        noop = mybir.InstNoOp(
                            name=f"{inst.name}-wsplit{i}",
                            engine=inst.engine,
                            ins=[],
                            outs=[],
                        )
                        noop.bass_nofuse = True
                        noop.sync_info = mybir.SyncInfo(on_wait=[wv], on_update=[])
                        new.append(noop)
                    inst.sync_info = mybir.SyncInfo(
                        on_wait=[waits[-1]], on_update=list(si.on_update)
                    )
                    n += 1
                new.append(inst)
            bb.instructions = new
    return n


def _build_program():
    import concourse.bass as bass
    import concourse.tile as tile
    from concourse import mybir

    f32 = mybir.dt.float32
    bf16 = mybir.dt.bfloat16
    f32r = mybir.dt.float32r
    AF = mybir.ActivationFunctionType

    nc = bass.Bass()

    # all pre-tiled on the host: leading dim = SBUF partition, per-partition
    # data contiguous in DRAM so every DMA is a few big descriptors
    hT_d = nc.declare_dram_parameter("hT", [P, T // CH, HID // P, CH], bf16, isOutput=False)
    wqa_d = nc.declare_dram_parameter("wqa", [P, QL // P, HID // P, P], bf16, isOutput=False)
    # latent 512 | ropeA dup 128 | ropeB dup 128
    wkva_d = nc.declare_dram_parameter("wkva", [P, (KVL + 256) // P, HID // P, P], bf16, isOutput=False)
    # h0_nope 128 | h1_nope 128 | ropeA 128 | ropeB 128  (ln folded)
    wqb_d = nc.declare_dram_parameter("wqb", [P, 4, QL // P, P], f32r, isOutput=False)
    wkvbk_d = nc.declare_dram_parameter("wkvbk", [P, KVL // P, HLOC * DN], f32r, isOutput=False)
    wkvbv_d = nc.declare_dram_parameter("wkvbv", [P, KVL // P, HLOC * DV], f32r, isOutput=False)
    wo_d = nc.declare_dram_parameter("wo", [P, HLOC, HID], f32r, isOutput=False)
    cos2_d = nc.declare_dram_parameter("cos2", [P, T], f32, isOutput=False)
    sin2_d = nc.declare_dram_parameter("sin2", [P, T], f32, isOutput=False)
    trimask_d = nc.declare_dram_parameter("trimask", [P, P], f32, isOutput=False)
    y_d = nc.declare_dram_parameter("y", [T, HID], f32, isOutput=True)

    hT3 = hT_d[:, :, :, :]
    wqa3 = wqa_d[:, :, :, :]
    wkva3 = wkva_d[:, :, :, :]
    wqb3 = wqb_d[:, :, :, :]
    wkvbk3 = wkvbk_d[:, :, :]
    wkvbv3 = wkvbv_d[:, :, :]
    wo3 = wo_d[:, :, :]

    NKQ = QL // P    # 12
    NKV = KVL // P   # 4

    def r32(ap):
        return ap.bitcast(f32r)

    with tile.TileContext(nc) as tc, nc.allow_low_precision(
        reason="fp32r rounding on PE-matmul operands is intentional"
    ):
        with tc.tile_pool(name="persist", bufs=1) as pp:
            # persistent SBUF tensors
            wkvbk_sb = pp.tile([P, NKV, HLOC * DN], f32r, name="wkvbk")
            nc.gpsimd.dma_start(out=wkvbk_sb, in_=wkvbk3)
            wkvbv_sb = pp.tile([P, NKV, HLOC * DV], f32r, name="wkvbv")
            nc.gpsimd.dma_start(out=wkvbv_sb, in_=wkvbv3)
            wo_sb = pp.tile([P, HLOC, T], f32r, name="wo")
            nc.gpsimd.dma_start(out=wo_sb, in_=wo3)
            trimask_sb = pp.tile([P, P], f32, name="trimask")
            nc.gpsimd.dma_start(out=trimask_sb, in_=trimask_d[:, :])
            ones_f = pp.tile([P, P], f32, name="ones_f")
            nc.vector.memset(ones_f, 1.0)
            ones_sb = pp.tile([P, 1], f32r, name="ones")
            nc.vector.tensor_copy(ones_sb, ones_f[:, 0:1])
            col_ones = pp.tile([1, P], f32r, name="col_ones")
            nc.vector.tensor_copy(col_ones, ones_f[0:1, :])
            zmask = pp.tile([P, HLOC], f32, name="zmask")
            nc.vector.memset(zmask[0:DR, 0:1], 1.0)
            nc.vector.memset(zmask[DR:P, 0:1], 0.0)
            nc.vector.memset(zmask[0:DR, 1:2], 0.0)
            nc.vector.memset(zmask[DR:P, 1:2], 1.0)
            eps_sb = pp.tile([1, 1], f32, name="eps")
            nc.vector.memset(eps_sb, EPS)

            qTn = [pp.tile([P, T], f32r, name=f"qTn{h}") for h in range(HLOC)]
            qpeT2 = pp.tile([P, T], f32r, name="qpeT2")   # h0 rope rows 0:64, h1 64:128
            KT = [pp.tile([P, T], f32r, name=f"KT{h}") for h in range(HLOC)]
            kpe2 = [pp.tile([P, T], f32r, name=f"kpe2{h}") for h in range(HLOC)]
            V_sb = [pp.tile([P, HLOC * DV], f32r, name=f"v{i}") for i in range(NKB)]

            # ---------------- Stage A: projections ----------------
            with (
                tc.tile_pool(name="achunk", bufs=1) as ap_,
                tc.tile_pool(name="astream", bufs=2) as sp_,
                tc.tile_pool(name="asmall", bufs=1) as smp,
                tc.tile_pool(name="aps", bufs=3, space="PSUM") as s1ps,
                tc.tile_pool(name="upps", bufs=3, space="PSUM") as upps,
                tc.tile_pool(name="ssqps", bufs=1, space="PSUM") as ssqps,
            ):
                for c in range(NCH):
                    t0 = c * CH
                    h_sb = ap_.tile([P, HID // P, CH], bf16, name="hchunk", bufs=2)
                    nc.sync.dma_start(out=h_sb, in_=hT3[:, c, :, :])
                    cos_sb = smp.tile([P, CH], f32, name="cosc")
                    nc.gpsimd.dma_start(out=cos_sb, in_=cos2_d[:, t0 : t0 + CH])
                    sin_sb = smp.tile([P, CH], f32, name="sinc")
                    nc.gpsimd.dma_start(out=sin_sb, in_=sin2_d[:, t0 : t0 + CH])

                    ssq_q = ssqps.tile([1, CH], f32, name="ssqq")
                    ssq_kv = ssqps.tile([1, CH], f32, name="ssqkv")

                    # q path stage-1: q_c^T[m] = wqa[:,m].T @ hidden^T
                    qc_sb = []
                    for m in range(NKQ):
                        wq_sb = sp_.tile([P, HID // P, P], bf16, name="wstream")
                        nc.sync.dma_start(out=wq_sb, in_=wqa3[:, m, :, :])
                        ps = s1ps.tile([P, CH], f32, name="s1")
                        for k in range(HID // P):
                            nc.tensor.matmul(
                                ps,
                                lhsT=wq_sb[:, k, :],
                                rhs=h_sb[:, k, :],
                                start=(k == 0),
                                stop=(k == HID // P - 1),
                            )
                        qt = ap_.tile([P, CH], f32r, name=f"qc{m}")
                        nc.vector.tensor_copy(qt, ps)
                        qc_sb.append(qt)
                        sq = smp.tile([P, CH], f32r, name="sq", bufs=1)
                        nc.scalar.square(sq, ps)
                        nc.tensor.matmul(
                            ssq_q,
                            lhsT=r32(ones_sb),
                            rhs=r32(sq),
                            start=(m == 0),
                            stop=(m == NKQ - 1),
                        )

                    # kv path stage-1: latent 4 tiles + ropeA/ropeB dup tiles
                    kva_sb = []
                    for m in range(NKV + 2):
                        wk_sb = sp_.tile([P, HID // P, P], bf16, name="wstream")
                        nc.sync.dma_start(out=wk_sb, in_=wkva3[:, m, :, :])
                        ps = s1ps.tile([P, CH], f32, name="s1")
                        for k in range(HID // P):
                            nc.tensor.matmul(
                                ps,
                                lhsT=wk_sb[:, k, :],
                                rhs=h_sb[:, k, :],
                                start=(k == 0),
                                stop=(k == HID // P - 1),
                            )
                        if m < NKV:
                            kt = ap_.tile([P, CH], f32r, name=f"kva{m}")
                            nc.vector.tensor_copy(kt, ps)
                            kva_sb.append(kt)
                            sq = smp.tile([P, CH], f32r, name="sq", bufs=1)
                            nc.scalar.square(sq, ps)
                            nc.tensor.matmul(
                                ssq_kv,
                                lhsT=r32(ones_sb),
                                rhs=r32(sq),
                                start=(m == 0),
                                stop=(m == NKV - 1),
                            )
                        else:
                            # rope A2/B2 stay in PSUM; rope muls read them there
                            kva_sb.append(ps)

                    # rms scales r = rsqrt(mean+eps) (ACT+DVE, overlaps PE)
                    rq = smp.tile([1, CH], f32r, name="rq")
                    nc.scalar.activation(
                        rq, ssq_q, func=AF.Sqrt, bias=eps_sb, scale=1.0 / QL
                    )
                    nc.vector.reciprocal(rq, rq)
                    rkv = smp.tile([1, CH], f32r, name="rkv")
                    nc.scalar.activation(
                        rkv, ssq_kv, func=AF.Sqrt, bias=eps_sb, scale=1.0 / KVL
                    )
                    nc.vector.reciprocal(rkv, rkv)

                    # k_pe rope first (DVE work that overlaps q up-proj;
                    # A2/B2 are read straight from their stage-1 PSUM tiles)
                    t1 = smp.tile([P, CH], f32, name="ropet1")
                    t2 = smp.tile([P, CH], f32, name="ropet2")
                    nc.vector.tensor_mul(t1, kva_sb[NKV], cos_sb)
                    nc.vector.tensor_mul(t2, kva_sb[NKV + 1], sin_sb)
                    nc.vector.tensor_add(t1, t1, t2)
                    for h in range(HLOC):
                        nc.vector.tensor_scalar_mul(
                            kpe2[h][:, t0 : t0 + CH], t1, zmask[:, h : h + 1]
                        )

                    # q up-proj mo=0,1 (independent of the rms scales -> PE
                    # keeps running while ACT/DVE produce rq/rkv)
                    ups = []
                    for mo in range(4):
                        if mo == 2:
                            # rms broadcast matmuls sit here: by now sqrt and
                            # reciprocal are long done, so PE doesn't stall
                            rqb_ps = s1ps.tile([P, CH], f32, name="s1")
                            nc.tensor.matmul(rqb_ps, lhsT=col_ones, rhs=rq, start=True, stop=True)
                            rq_b = smp.tile([P, CH], f32, name="rqb")
                            nc.vector.tensor_copy(rq_b, rqb_ps)
                            rkvb_ps = s1ps.tile([P, CH], f32, name="s1")
                            nc.tensor.matmul(rkvb_ps, lhsT=col_ones, rhs=rkv, start=True, stop=True)
                            rkv_b = smp.tile([P, CH], f32, name="rkvb")
                            nc.vector.tensor_copy(rkv_b, rkvb_ps)
                            # normalize kv_c latent in place (DVE, before the
                            # kv up-proj matmuls read it)
                            for m in range(NKV):
                                nc.vector.tensor_mul(kva_sb[m], kva_sb[m], rkv_b)
                        wqbs = sp_.tile([P, NKQ, P], f32r, name="wqbs")
                        nc.gpsimd.dma_start(out=wqbs, in_=wqb3[:, mo, :, :])
                        ps = upps.tile([P, CH], f32, name="up")
                        for k in range(NKQ):
                            nc.tensor.matmul(
                                ps,
                                lhsT=wqbs[:, k, :],
                                rhs=r32(qc_sb[k]),
                                start=(k == 0),
                                stop=(k == NKQ - 1),
                            )
                        ups.append(ps)
                    # nope heads: multiply by rms scale on copy-out
                    for h in range(HLOC):
                        nc.vector.tensor_mul(
                            qTn[h][:, t0 : t0 + CH], ups[h], rq_b
                        )
                    # rope: (A*cos + B*sin) * r
                    t3 = smp.tile([P, CH], f32, name="ropet1")
                    t4 = smp.tile([P, CH], f32, name="ropet2")
                    nc.vector.tensor_mul(t3, ups[2], cos_sb)
                    nc.vector.tensor_mul(t4, ups[3], sin_sb)
                    nc.vector.tensor_add(t3, t3, t4)
                    nc.vector.tensor_mul(qpeT2[:, t0 : t0 + CH], t3, rq_b)

                    # kv up-projection: K^T per head
                    for mo in range(HLOC):
                        ps = upps.tile([P, CH], f32, name="up")
                        for k in range(NKV):
                            nc.tensor.matmul(
                                ps,
                                lhsT=r32(wkvbk_sb[:, k, mo * P : (mo + 1) * P]),
                                rhs=r32(kva_sb[k]),
                                start=(k == 0),
                                stop=(k == NKV - 1),
                            )
                        nc.vector.tensor_copy(KT[mo][:, t0 : t0 + CH], ps)
                    # V: natural orientation [t, dv*2]
                    for tt in range(CH // P):
                        ps = upps.tile([P, HLOC * DV], f32, name="up")
                        for k in range(NKV):
                            nc.tensor.matmul(
                                ps,
                                lhsT=r32(kva_sb[k][:, tt * P : (tt + 1) * P]),
                                rhs=r32(wkvbv_sb[:, k, :]),
                                start=(k == 0),
                                stop=(k == NKV - 1),
                            )
                        nc.vector.tensor_copy(V_sb[(t0 // P) + tt], ps)

            # ---------------- Stage B: attention ----------------
            with (
                tc.tile_pool(name="bpt", bufs=4) as ptp,
                tc.tile_pool(name="bsmall", bufs=3) as bsm,
                tc.tile_pool(name="sps", bufs=2, space="PSUM") as spsp,
                tc.tile_pool(name="otps", bufs=2, space="PSUM") as otpsp,
                tc.tile_pool(name="lps", bufs=2, space="PSUM") as lpsp,
            ):
                OT_sb = [
                    [ptp.tile([P, QC], f32r, name=f"ot{h}_{j}", bufs=1) for j in range(NQC)]
                    for h in range(HLOC)
                ]
                def flush_norm(pend):
                    p_ot, p_l, p_h, p_j = pend
                    recl = bsm.tile([1, QC], f32r, name="recl")
                    nc.vector.reciprocal(recl, p_l)
                    lb_ps = spsp.tile([P, 2 * QC], f32, name="sps2")[:, :QC]
                    nc.tensor.matmul(lb_ps, lhsT=col_ones, rhs=recl, start=True, stop=True)
                    lb = bsm.tile([P, QC], f32, name="lb")
                    nc.scalar.copy(lb, lb_ps)
                    nc.vector.tensor_mul(OT_sb[p_h][p_j], p_ot, lb)

                pend = None
                for h in range(HLOC):
                    for j in range(NQC):
                        ot_ps = otpsp.tile([P, QC], f32, name="otps")
                        l_ps = lpsp.tile([1, QC], f32, name="lps")
                        nkb = 4 * (j + 1)
                        qcol0 = j * QC
                        for kp in range(0, nkb, 2):
                            # two k-blocks share one PSUM pair and ONE wide exp
                            # (ACT per-op overhead halved); scores are computed
                            # full-width, PV/denominator still column-clipped
                            s2 = spsp.tile([P, 2 * QC], f32, name="sps2")
                            for u in range(2):
                                ki = kp + u
                                nc.tensor.matmul(
                                    s2[:, u * QC : (u + 1) * QC],
                                    lhsT=r32(KT[h][:, ki * P : (ki + 1) * P]),
                                    rhs=r32(qTn[h][:, qcol0 : qcol0 + QC]),
                                    start=True,
                                    stop=False,
                                )
                                nc.tensor.matmul(
                                    s2[:, u * QC : (u + 1) * QC],
                                    lhsT=r32(kpe2[h][:, ki * P : (ki + 1) * P]),
                                    rhs=r32(qpeT2[:, qcol0 : qcol0 + QC]),
                                    start=False,
                                    stop=True,
                                )
                            pt = ptp.tile([P, 2 * QC], f32r, name="pt")
                            nc.scalar.activation(pt, s2, func=AF.Exp, scale=SCALE)
                            for u in range(2):
                                ki = kp + u
                                diag = (ki // 4 == j)
                                cs = (ki % 4) * P if diag else 0
                                W = QC - cs
                                if diag:
                                    nc.gpsimd.tensor_mul(
                                        pt[:, u * QC + cs : u * QC + cs + P],
                                        pt[:, u * QC + cs : u * QC + cs + P],
                                        trimask_sb,
                                    )
                                nc.tensor.matmul(
                                    ot_ps[:, cs:],
                                    lhsT=r32(V_sb[ki][:, h * DV : (h + 1) * DV]),
                                    rhs=r32(pt[:, u * QC + cs : (u + 1) * QC]),
                                    start=(ki == 0),
                                    stop=(ki == nkb - 1),
                                )
                                nc.tensor.matmul(
                                    l_ps[:, cs:],
                                    lhsT=r32(ones_sb),
                                    rhs=r32(pt[:, u * QC + cs : (u + 1) * QC]),
                                    start=(ki == 0),
                                    stop=(ki == nkb - 1),
                                )
                            if kp == 2 and pend is not None:
                                # normalize the PREVIOUS (h,j): by now its
                                # reciprocal is done, the broadcast matmul
                                # won't stall the in-order PE queue
                                flush_norm(pend)
                                pend = None
                        pend = (ot_ps, l_ps, h, j)
                flush_norm(pend)

                # ---------------- o_proj ----------------
                for tt in range(T // P):
                    j, sub = tt // 4, (tt % 4) * P
                    for n in range(HID // QC):
                        y_ps = spsp.tile([P, 2 * QC], f32, name="sps2")[:, :QC]
                        for h in range(HLOC):
                            nc.tensor.matmul(
                                y_ps,
                                lhsT=r32(OT_sb[h][j][:, sub : sub + P]),
                                rhs=r32(wo_sb[:, h, n * QC : (n + 1) * QC]),
                                start=(h == 0),
                                stop=(h == HLOC - 1),
                            )
                        y_sb = ptp.tile([P, QC], f32, name="ysb")
                        nc.scalar.copy(y_sb, y_ps)
                        nc.sync.dma_start(
                            out=y_d[tt * P : (tt + 1) * P, n * QC : (n + 1) * QC],
                            in_=y_sb,
                        )
    _split_multi_waits(nc, mybir)
    return nc


def _host_prep(inputs):
    import ml_dtypes

    hs = np.ascontiguousarray(np.asarray(inputs["hidden_states"], np.float32))
    pos = np.asarray(inputs["positions"], np.int32)
    w_qa = np.asarray(inputs["w_qa"], np.float32)
    q_ln = np.asarray(inputs["q_a_ln_w"], np.float32)
    w_qb = np.asarray(inputs["w_qb"], np.float32)
    w_kva = np.asarray(inputs["w_kva"], np.float32)
    kv_ln = np.asarray(inputs["kv_a_ln_w"], np.float32)
    w_kvb = np.asarray(inputs["w_kvb"], np.float32)
    w_o = np.asarray(inputs["w_o"], np.float32)

    bf = ml_dtypes.bfloat16
    # pre-tiled: [p, c, k, t] with per-partition contiguous (k, t)
    hT = np.ascontiguousarray(
        hs.reshape(T // CH, CH, HID // P, P).transpose(3, 0, 2, 1)
    ).astype(bf)
    wqa_b = np.ascontiguousarray(
        w_qa.reshape(HID // P, P, QL // P, P).transpose(1, 2, 0, 3)
    ).astype(bf)

    # rope tables (neox): match the f32 reference computation
    inv_freq = (1.0 / (10000.0 ** (np.arange(0, DR, 2, dtype=np.float32) / DR))).astype(
        np.float32
    )
    freqs = pos.astype(np.float32)[:, None] * inv_freq[None, :]
    emb = np.concatenate([freqs, freqs], axis=-1)  # [T, 64]
    cosT = np.ascontiguousarray(np.cos(emb).T.astype(np.float32))  # [64, T]
    sinT = np.ascontiguousarray(np.sin(emb).T.astype(np.float32))
    cos2 = np.ascontiguousarray(np.concatenate([cosT, cosT], axis=0))  # [128, T]
    sin2 = np.ascontiguousarray(np.concatenate([sinT, sinT], axis=0))

    def rot_cols(A):
        # columns of rot(x): d<32 -> -col(d+32); d>=32 -> +col(d-32)
        return np.concatenate([-A[:, DR // 2 :], A[:, : DR // 2]], axis=1)

    # kv a-projection augmented with duplicated rope A/B columns
    kva_lat = w_kva[:, :KVL]
    kva_rope = w_kva[:, KVL:]                       # [2048, 64]
    kva_ropeB = rot_cols(kva_rope)
    wkva_aug = np.concatenate(
        [kva_lat, kva_rope, kva_rope, kva_ropeB, kva_ropeB], axis=1
    )  # [2048, 512+128+128]
    wkva_b = np.ascontiguousarray(
        wkva_aug.reshape(HID // P, P, 6, P).transpose(1, 2, 0, 3)
    ).astype(bf)

    w_qb_f = (w_qb * q_ln[:, None]).reshape(QL, H, DQK)
    w_kvb_f = (w_kvb * kv_ln[:, None]).reshape(KVL, H, DN + DV)
    w_o_r = w_o.reshape(H, DV, HID)

    trimask = np.triu(np.ones((P, P), dtype=np.float32))  # [k, q]: 1 iff q>=k

    per_core = []
    for i in range(NC_):
        hh = [HLOC * i + x for x in range(HLOC)]
        nope = np.concatenate([w_qb_f[:, h, :DN] for h in hh], axis=1)  # [QL,256]
        ropeA = np.concatenate([w_qb_f[:, h, DN:] for h in hh], axis=1)  # [QL,128]
        ropeB = np.concatenate(
            [rot_cols(w_qb_f[:, h, DN:]) for h in hh], axis=1
        )
        wqb_aug = np.ascontiguousarray(
            np.concatenate([nope, ropeA, ropeB], axis=1)
            .reshape(QL // P, P, 4, P)
            .transpose(1, 2, 0, 3)
        )  # [p, mo, k, j]
        wkvbk = np.ascontiguousarray(
            np.concatenate([w_kvb_f[:, h, :DN] for h in hh], axis=1)
            .reshape(KVL // P, P, HLOC * DN)
            .transpose(1, 0, 2)
        )
        wkvbv = np.ascontiguousarray(
            np.concatenate([w_kvb_f[:, h, DN:] for h in hh], axis=1)
            .reshape(KVL // P, P, HLOC * DV)
            .transpose(1, 0, 2)
        )
        wo_i = np.ascontiguousarray(
            np.stack([w_o_r[h] for h in hh], axis=0).transpose(1, 0, 2)
        )  # [p, h, HID]
        per_core.append(
            dict(
                hT=hT,
                wqa=wqa_b,
                wkva=wkva_b,
                wqb=wqb_aug,
                wkvbk=wkvbk,
                wkvbv=wkvbv,
                wo=wo_i,
                cos2=cos2,
                sin2=sin2,
                trimask=trimask,
            )
        )
    return per_core


def kernel(**inputs):
    global LAST_RESULTS
    from concourse.bass_utils import run_bass_kernel_spmd

    if "nc" not in _CACHE:
        _CACHE["nc"] = _build_program()
    nc = _CACHE["nc"]

    in_maps = _host_prep(inputs)
    res = run_bass_kernel_spmd(nc, in_maps, core_ids=list(range(NC_)))
    LAST_RESULTS = res
    out = np.zeros((T, HID), dtype=np.float32)
    for r in res.results:
        out += np.asarray(r["y"], dtype=np.float32)
    return out



# revision 8
# speedup vs baseline: 1.3067x; 1.3067x over previous
"""DeepseekV2 MLA attention (T=2048, H=16) on 8 trn2 cores.

Sharding v2 (collective-based, no replicated stage-1):
- Stage 1 (the big a-projections) is sequence-sharded: each core computes
  q_c / kv latent / roped k_pe for its own 256 tokens only.
- The q up-projection stays sequence-sharded (own tokens, ALL 16 heads,
  rms scale folded in); an AllToAll then redistributes q by head-pair so
  each core ends with its 2 heads over the full sequence.
- The kv latent (+ roped k_pe) is AllGather'd (it is tiny: that is the
  point of MLA), and each core up-projects K/V for its 2 heads locally.
- A dummy collective issued at t=0 absorbs the one-time ~70us comm-init
  cost while stage 1 runs on the PE.
- Attention itself is head-sharded exactly like the baseline: scores as
  S^T = K^T q blocks, no row-max (logits are ~N(0,1)), denominator via
  ones-matmul, normalization after P@V, causal diag via a 0/1 tri mask
  applied post-exp, o_proj partials summed on the host.
- Everything on the PE runs in bf16 (1 cyc/row); PSUM accumulation is
  f32.
"""

import numpy as np

T = 2048
HID = 2048
H = 16
NC_ = 8
HLOC = H // NC_          # 2 heads per core
NP = H // HLOC           # 8 head-pairs
QL = 1536                # q lora
KVL = 512                # kv lora
DN = 128                 # nope dim
DR = 64                  # rope dim
DQK = DN + DR            # 192
DV = 128
EPS = 1e-6
SCALE = float(DQK) ** -0.5
P = 128
TC = T // NC_            # 256 tokens per core (stage-1 shard)
NKQ = QL // P            # 12
NKV = KVL // P           # 4
NKH = HID // P           # 16
QC = 512                 # attention q-chunk
NQC = T // QC
NKB = T // P             # key blocks

_CACHE = {}
LAST_RESULTS = None


def _split_multi_waits(nc, mybir):
    """Walrus embeds at most one sem/event wait per TPB instruction; hoist
    extra waits onto preceding same-engine NoOps (queue FIFO keeps order)."""
    n = 0
    for f in nc.m.functions:
        for bb in f.blocks:
            new = []
            for inst in bb.instructions:
                si = getattr(inst, "sync_info", None)
                if si is not None and len(si.on_wait) > 1:
                    waits = list(si.on_wait)
                    for i, wv in enumerate(waits[:-1]):
                        noop = mybir.InstNoOp(
                            name=f"{inst.name}-wsplit{i}",
                            engine=inst.engine,
                            ins=[],
                            outs=[],
                        )
                        noop.bass_nofuse = True
                        noop.sync_info = mybir.SyncInfo(on_wait=[wv], on_update=[])
                        new.append(noop)
                    inst.sync_info = mybir.SyncInfo(
                        on_wait=[waits[-1]], on_update=list(si.on_update)
                    )
                    n += 1
                new.append(inst)
            bb.instructions = new
    return n


def _build_program():
    import concourse.bass as bass
    import concourse.tile as tile
    from concourse import mybir

    f32 = mybir.dt.float32
    bf16 = mybir.dt.bfloat16
    f32r = mybir.dt.float32r
    AF = mybir.ActivationFunctionType
    GRP = [list(range(NC_))]

    nc = bass.Bass(num_devices=NC_)

    # ---- dram parameters (per-core values supplied by the host) ----
    # own 256-token hidden chunk, transposed+tiled
    hT_d = nc.declare_dram_parameter("hT", [P, NKH, TC], bf16, isOutput=False)
    # full a-proj weights (replicated): latent 512 | ropeA dup 128 | ropeB dup 128
    wqa_d = nc.declare_dram_parameter("wqa", [P, NKQ, NKH, P], bf16, isOutput=False)
    wkva_d = nc.declare_dram_parameter("wkva", [P, NKV + 2, NKH, P], bf16, isOutput=False)
    # full q b-projection for ALL head-pairs: [p, pair, mo, k, col]
    # mo: 0/1 = nope h0/h1, 2 = ropeA dup, 3 = ropeB dup  (ln folded)
    wqb_d = nc.declare_dram_parameter("wqb", [P, NP, 4, NKQ, P], bf16, isOutput=False)
    # kv b-projection for OWN 2 heads only
    wkvbk_d = nc.declare_dram_parameter("wkvbk", [P, NKV, HLOC * DN], bf16, isOutput=False)
    wkvbv_d = nc.declare_dram_parameter("wkvbv", [P, NKV, HLOC * DV], bf16, isOutput=False)
    wo_d = nc.declare_dram_parameter("wo", [P, HLOC, HID], bf16, isOutput=False)
    # rope tables: local chunk in f32 (k_pe), full-T in bf16 (q_pe)
    cosl_d = nc.declare_dram_parameter("cosl", [P, TC], f32, isOutput=False)
    sinl_d = nc.declare_dram_parameter("sinl", [P, TC], f32, isOutput=False)
    cosf_d = nc.declare_dram_parameter("cosf", [P, T], bf16, isOutput=False)
    sinf_d = nc.declare_dram_parameter("sinf", [P, T], bf16, isOutput=False)
    trimask_d = nc.declare_dram_parameter("trimask", [P, P], bf16, isOutput=False)
    y_d = nc.declare_dram_parameter("y", [T, HID], f32, isOutput=True)

    # ---- dram bounce buffers for the collectives ----
    d_in = nc.dram_tensor("d_in", [P, 1], f32)
    d_out = nc.dram_tensor("d_out", [NC_, P, 1], f32, addr_space="Shared")
    # kv payload: 4 latent tiles + 1 roped kpe (dup) tile, [p, m, t]
    kv_in = nc.dram_tensor("kv_in", [P, NKV + 1, TC], bf16)
    kv_out = nc.dram_tensor(
        "kv_out", [NC_, P, NKV + 1, TC], bf16, addr_space="Shared"
    )
    # q payload: per dst head-pair [p, mo, t] (partition-major like SBUF)
    q_in = nc.dram_tensor("q_in", [NP, P, 4, TC], bf16)
    q_out = nc.dram_tensor("q_out", [NC_, P, 4, TC], bf16)

    with tile.TileContext(nc) as tc, nc.allow_low_precision(
        reason="bf16 matmul operands are intentional"
    ):
        # dummy collective: warms up the comm channel during stage-1
        nc.gpsimd.collective_compute(
            "AllGather",
            mybir.AluOpType.bypass,
            replica_groups=GRP,
            ins=[d_in[:, :].opt()],
            outs=[d_out[:, :, :].opt()],
        )

        with tc.tile_pool(name="persist", bufs=1) as pp:
            # ---- persistent SBUF ----
            wkvbk_sb = pp.tile([P, NKV, HLOC * DN], bf16, name="wkvbk")
            nc.sync.dma_start(out=wkvbk_sb, in_=wkvbk_d[:, :, :])
            wkvbv_sb = pp.tile([P, NKV, HLOC * DV], bf16, name="wkvbv")
            nc.sync.dma_start(out=wkvbv_sb, in_=wkvbv_d[:, :, :])
            wo_sb = pp.tile([P, HLOC, HID], bf16, name="wo")
            nc.sync.dma_start(out=wo_sb, in_=wo_d[:, :, :])
            trimask_sb = pp.tile([P, P], bf16, name="trimask")
            nc.sync.dma_start(out=trimask_sb, in_=trimask_d[:, :])
            cosl_sb = pp.tile([P, TC], f32, name="cosl")
            nc.sync.dma_start(out=cosl_sb, in_=cosl_d[:, :])
            sinl_sb = pp.tile([P, TC], f32, name="sinl")
            nc.sync.dma_start(out=sinl_sb, in_=sinl_d[:, :])
            cosf_sb = pp.tile([P, T], bf16, name="cosf")
            nc.sync.dma_start(out=cosf_sb, in_=cosf_d[:, :])
            sinf_sb = pp.tile([P, T], bf16, name="sinf")
            nc.sync.dma_start(out=sinf_sb, in_=sinf_d[:, :])
            h_sb = pp.tile([P, NKH, TC], bf16, name="hchunk")
            nc.sync.dma_start(out=h_sb, in_=hT_d[:, :, :])

            ones_f = pp.tile([P, P], f32, name="ones_f")
            nc.vector.memset(ones_f, 1.0)
            ones_sb = pp.tile([P, 1], f32r, name="ones")
            nc.vector.tensor_copy(ones_sb, ones_f[:, 0:1])
            ones_bf = pp.tile([P, 1], bf16, name="ones_bf")
            nc.vector.tensor_copy(ones_bf, ones_f[:, 0:1])
            col_ones = pp.tile([1, P], f32r, name="col_ones")
            nc.vector.tensor_copy(col_ones, ones_f[0:1, :])
            zmask = pp.tile([P, HLOC], f32, name="zmask")
            nc.vector.memset(zmask[0:DR, 0:1], 1.0)
            nc.vector.memset(zmask[DR:P, 0:1], 0.0)
            nc.vector.memset(zmask[0:DR, 1:2], 0.0)
            nc.vector.memset(zmask[DR:P, 1:2], 1.0)
            eps_sb = pp.tile([1, 1], f32, name="eps")
            nc.vector.memset(eps_sb, EPS)

            qc_sb = pp.tile([P, NKQ, TC], bf16, name="qc")
            pay_kv = pp.tile([P, NKV + 1, TC], bf16, name="paykv")
            pay_q = pp.tile([P, NP * 4, TC], bf16, name="payq")
            rq_b = pp.tile([P, TC], bf16, name="rqb")
            rkv_b = pp.tile([P, TC], bf16, name="rkvb")

            qTn = [pp.tile([P, T], bf16, name=f"qTn{h}") for h in range(HLOC)]
            qpeT2 = pp.tile([P, T], bf16, name="qpeT2")
            ropeA_f = pp.tile([P, T], bf16, name="ropeAf")
            ropeB_f = pp.tile([P, T], bf16, name="ropeBf")
            KT = [pp.tile([P, T], bf16, name=f"KT{h}") for h in range(HLOC)]
            kpe_raw = pp.tile([P, T], bf16, name="kperaw")
            kpe2 = [pp.tile([P, T], bf16, name=f"kpe2{h}") for h in range(HLOC)]
            kvn_sb = pp.tile([P, NKV, T], bf16, name="kvn")
            V_sb = [pp.tile([P, HLOC * DV], bf16, name=f"v{i}") for i in range(NKB)]

            # ---------------- Stage A: sharded projections ----------------
            with (
                tc.tile_pool(name="astream", bufs=2) as sp_,
                tc.tile_pool(name="aqbstream", bufs=3) as qbp,
                tc.tile_pool(name="asmall", bufs=1) as smp,
                tc.tile_pool(name="aps", bufs=2, space="PSUM") as s1ps,
                tc.tile_pool(name="arope", bufs=1, space="PSUM") as rps,
                tc.tile_pool(name="ssqps", bufs=1, space="PSUM") as ssqps,
                tc.tile_pool(name="upps", bufs=2, space="PSUM") as upps,
            ):
                ssq_kv = ssqps.tile([1, TC], f32, name="ssqkv")
                ssq_q = ssqps.tile([1, TC], f32, name="ssqq")

                # --- kv path first (its payload gates CC#1) ---
                rope_ps = []
                for m in range(NKV + 2):
                    wk_sb = sp_.tile([P, NKH, P], bf16, name="wstream")
                    nc.sync.dma_start(out=wk_sb, in_=wkva_d[:, m, :, :])
                    if m < NKV:
                        ps = s1ps.tile([P, TC], f32, name="s1")
                    else:
                        ps = rps.tile([P, TC], f32, name=f"rope{m - NKV}")
                    for k in range(NKH):
                        nc.tensor.matmul(
                            ps,
                            lhsT=wk_sb[:, k, :],
                            rhs=h_sb[:, k, :],
                            start=(k == 0),
                            stop=(k == NKH - 1),
                        )
                    if m < NKV:
                        nc.vector.tensor_copy(pay_kv[:, m, :], ps)
                        sq = smp.tile([P, TC], f32r, name="sq", bufs=1)
                        nc.scalar.square(sq, ps)
                        nc.tensor.matmul(
                            ssq_kv,
                            lhsT=ones_sb,
                            rhs=sq,
                            start=(m == 0),
                            stop=(m == NKV - 1),
                        )
                    else:
                        rope_ps.append(ps)

                # rkv scale + broadcast
                rkv = smp.tile([1, TC], f32r, name="rkv")
                nc.scalar.activation(
                    rkv, ssq_kv, func=AF.Sqrt, bias=eps_sb, scale=1.0 / KVL
                )
                nc.vector.reciprocal(rkv, rkv)
                rkvb_ps = upps.tile([P, TC], f32, name="up")
                nc.tensor.matmul(rkvb_ps, lhsT=col_ones, rhs=rkv, start=True, stop=True)
                nc.vector.tensor_copy(rkv_b, rkvb_ps)
                # roped k_pe (dup rows) in f32, then normalize latent payload
                t1 = smp.tile([P, TC], f32, name="ropet1")
                t2 = smp.tile([P, TC], f32, name="ropet2")
                nc.vector.tensor_mul(t1, rope_ps[0], cosl_sb)
                nc.vector.tensor_mul(t2, rope_ps[1], sinl_sb)
                nc.vector.tensor_add(pay_kv[:, NKV, :], t1, t2)
                for m in range(NKV):
                    nc.vector.tensor_mul(pay_kv[:, m, :], pay_kv[:, m, :], rkv_b)
                nc.sync.dma_start(out=kv_in[:, :, :], in_=pay_kv)  # [p,m,t] both
                nc.gpsimd.collective_compute(
                    "AllGather",
                    mybir.AluOpType.bypass,
                    replica_groups=GRP,
                    ins=[kv_in[:, :, :].opt()],
                    outs=[kv_out[:, :, :, :].opt()],
                )

                # --- q path stage-1 ---
                for m in range(NKQ):
                    wq_sb = sp_.tile([P, NKH, P], bf16, name="wstream")
                    nc.sync.dma_start(out=wq_sb, in_=wqa_d[:, m, :, :])
                    ps = s1ps.tile([P, TC], f32, name="s1")
                    for k in range(NKH):
                        nc.tensor.matmul(
                            ps,
                            lhsT=wq_sb[:, k, :],
                            rhs=h_sb[:, k, :],
                            start=(k == 0),
                            stop=(k == NKH - 1),
                        )
                    nc.vector.tensor_copy(qc_sb[:, m, :], ps)
                    sq = smp.tile([P, TC], f32r, name="sq", bufs=1)
                    nc.scalar.square(sq, ps)
                    nc.tensor.matmul(
                        ssq_q,
                        lhsT=ones_sb,
                        rhs=sq,
                        start=(m == 0),
                        stop=(m == NKQ - 1),
                    )
                rq = smp.tile([1, TC], f32r, name="rq")
                nc.scalar.activation(
                    rq, ssq_q, func=AF.Sqrt, bias=eps_sb, scale=1.0 / QL
                )
                nc.vector.reciprocal(rq, rq)
                rqb_ps = upps.tile([P, TC], f32, name="up")
                nc.tensor.matmul(rqb_ps, lhsT=col_ones, rhs=rq, start=True, stop=True)
                nc.vector.tensor_copy(rq_b, rqb_ps)

                # --- q up-projection: own tokens, ALL head-pairs ---
                for p_ in range(NP):
                    for mo in range(4):
                        wqbs = qbp.tile([P, NKQ, P], bf16, name="wqbs")
                        nc.sync.dma_start(out=wqbs, in_=wqb_d[:, p_, mo, :, :])
                        ps = upps.tile([P, TC], f32, name="up")
                        for k in range(NKQ):
                            nc.tensor.matmul(
                                ps,
                                lhsT=wqbs[:, k, :],
                                rhs=qc_sb[:, k, :],
                                start=(k == 0),
                                stop=(k == NKQ - 1),
                            )
                        nc.vector.tensor_mul(pay_q[:, p_ * 4 + mo, :], ps, rq_b)
                    nc.sync.dma_start(
                        out=q_in[p_, :, :, :],
                        in_=pay_q[:, p_ * 4 : p_ * 4 + 4, :],
                    )  # [p, mo, t] both
                nc.gpsimd.collective_compute(
                    "AllToAll",
                    mybir.AluOpType.bypass,
                    replica_groups=GRP,
                    ins=[q_in[:, :, :, :].opt()],
                    outs=[q_out[:, :, :, :].opt()],
                )

            # ---------------- Stage B: gather-side compute ----------------
            with (
                tc.tile_pool(name="bpt", bufs=4) as ptp,
                tc.tile_pool(name="bsmall", bufs=3) as bsm,
                tc.tile_pool(name="sps", bufs=2, space="PSUM") as spsp,
                tc.tile_pool(name="otps", bufs=2, space="PSUM") as otpsp,
                tc.tile_pool(name="lps", bufs=2, space="PSUM") as lpsp,
            ):
                # kv readback + K/V up-projection for own heads
                for r in range(NC_):
                    nc.sync.dma_start(
                        out=kvn_sb[:, :, r * TC : (r + 1) * TC],
                        in_=kv_out[r, :, 0:NKV, :],
                    )
                    nc.sync.dma_start(
                        out=kpe_raw[:, r * TC : (r + 1) * TC],
                        in_=kv_out[r, :, NKV, :],
                    )
                for h in range(HLOC):
                    nc.vector.tensor_scalar_mul(
                        kpe2[h], kpe_raw, zmask[:, h : h + 1]
                    )
                for h in range(HLOC):
                    for j in range(T // QC):
                        ps = otpsp.tile([P, QC], f32, name="otps")
                        for k in range(NKV):
                            nc.tensor.matmul(
                                ps,
                                lhsT=wkvbk_sb[:, k, h * P : (h + 1) * P],
                                rhs=kvn_sb[:, k, j * QC : (j + 1) * QC],
                                start=(k == 0),
                                stop=(k == NKV - 1),
                            )
                        nc.vector.tensor_copy(KT[h][:, j * QC : (j + 1) * QC], ps)
                for tt in range(NKB):
                    ps = otpsp.tile([P, QC], f32, name="otps")[:, : HLOC * DV]
                    for k in range(NKV):
                        nc.tensor.matmul(
                            ps,
                            lhsT=kvn_sb[:, k, tt * P : (tt + 1) * P],
                            rhs=wkvbv_sb[:, k, :],
                            start=(k == 0),
                            stop=(k == NKV - 1),
                        )
                    nc.vector.tensor_copy(V_sb[tt], ps)

                # q readback + rope combine
                for r in range(NC_):
                    nc.sync.dma_start(
                        out=qTn[0][:, r * TC : (r + 1) * TC], in_=q_out[r, :, 0, :]
                    )
                    nc.sync.dma_start(
                        out=qTn[1][:, r * TC : (r + 1) * TC], in_=q_out[r, :, 1, :]
                    )
                    nc.sync.dma_start(
                        out=ropeA_f[:, r * TC : (r + 1) * TC], in_=q_out[r, :, 2, :]
                    )
                    nc.sync.dma_start(
                        out=ropeB_f[:, r * TC : (r + 1) * TC], in_=q_out[r, :, 3, :]
                    )
                t3 = bsm.tile([P, T], f32, name="qropet1", bufs=1)
                t4 = bsm.tile([P, T], f32, name="qropet2", bufs=1)
                nc.vector.tensor_mul(t3, ropeA_f, cosf_sb)
                nc.vector.tensor_mul(t4, ropeB_f, sinf_sb)
                nc.vector.tensor_add(qpeT2, t3, t4)

                # ---------------- attention ----------------
                OT_sb = [
                    [ptp.tile([P, QC], bf16, name=f"ot{h}_{j}", bufs=1) for j in range(NQC)]
                    for h in range(HLOC)
                ]

                def flush_norm(pend):
                    p_ot, p_l, p_h, p_j = pend
                    recl = bsm.tile([1, QC], f32r, name="recl")
                    nc.vector.reciprocal(recl, p_l)
                    lb_ps = spsp.tile([P, 2 * QC], f32, name="sps2")[:, :QC]
                    nc.tensor.matmul(lb_ps, lhsT=col_ones, rhs=recl, start=True, stop=True)
                    lb = bsm.tile([P, QC], f32, name="lb")
                    nc.scalar.copy(lb, lb_ps)
                    nc.vector.tensor_mul(OT_sb[p_h][p_j], p_ot, lb)

                pend = None
                for h in range(HLOC):
                    for j in range(NQC):
                        ot_ps = otpsp.tile([P, QC], f32, name="otps")
                        l_ps = lpsp.tile([1, QC], f32, name="lps")
                        nkb = 4 * (j + 1)
                        qcol0 = j * QC
                        for kp in range(0, nkb, 2):
                            # two k-blocks share one PSUM pair and ONE wide exp
                            s2 = spsp.tile([P, 2 * QC], f32, name="sps2")
                            for u in range(2):
                                ki = kp + u
                                nc.tensor.matmul(
                                    s2[:, u * QC : (u + 1) * QC],
                                    lhsT=KT[h][:, ki * P : (ki + 1) * P],
                                    rhs=qTn[h][:, qcol0 : qcol0 + QC],
                                    start=True,
                                    stop=False,
                                )
                                nc.tensor.matmul(
                                    s2[:, u * QC : (u + 1) * QC],
                                    lhsT=kpe2[h][:, ki * P : (ki + 1) * P],
                                    rhs=qpeT2[:, qcol0 : qcol0 + QC],
                                    start=False,
                                    stop=True,
                                )
                            pt = ptp.tile([P, 2 * QC], bf16, name="pt")
                            nc.scalar.activation(pt, s2, func=AF.Exp, scale=SCALE)
                            for u in range(2):
                                ki = kp + u
                                diag = (ki // 4 == j)
                                cs = (ki % 4) * P if diag else 0
                                if diag:
                                    nc.gpsimd.tensor_mul(
                                        pt[:, u * QC + cs : u * QC + cs + P],
                                        pt[:, u * QC + cs : u * QC + cs + P],
                                        trimask_sb,
                                    )
                                nc.tensor.matmul(
                                    ot_ps[:, cs:],
                                    lhsT=V_sb[ki][:, h * DV : (h + 1) * DV],
                                    rhs=pt[:, u * QC + cs : (u + 1) * QC],
                                    start=(ki == 0),
                                    stop=(ki == nkb - 1),
                                )
                                nc.tensor.matmul(
                                    l_ps[:, cs:],
                                    lhsT=ones_bf,
                                    rhs=pt[:, u * QC + cs : (u + 1) * QC],
                                    start=(ki == 0),
                                    stop=(ki == nkb - 1),
                                )
                            if kp == 2 and pend is not None:
                                flush_norm(pend)
                                pend = None
                        pend = (ot_ps, l_ps, h, j)
                flush_norm(pend)

                # ---------------- o_proj ----------------
                for tt in range(T // P):
                    j, sub = tt // 4, (tt % 4) * P
                    for n in range(HID // QC):
                        y_ps = spsp.tile([P, 2 * QC], f32, name="sps2")[:, :QC]
                        for h in range(HLOC):
                            nc.tensor.matmul(
                                y_ps,
                                lhsT=OT_sb[h][j][:, sub : sub + P],
                                rhs=wo_sb[:, h, n * QC : (n + 1) * QC],
                                start=(h == 0),
                                stop=(h == HLOC - 1),
                            )
                        y_sb = ptp.tile([P, QC], f32, name="ysb")
                        nc.vector.tensor_copy(y_sb, y_ps)
                        nc.sync.dma_start(
                            out=y_d[tt * P : (tt + 1) * P, n * QC : (n + 1) * QC],
                            in_=y_sb,
                        )
    _split_multi_waits(nc, mybir)
    return nc


def _host_prep(inputs):
    import ml_dtypes

    bf = ml_dtypes.bfloat16
    hs = np.ascontiguousarray(np.asarray(inputs["hidden_states"], np.float32))
    pos = np.asarray(inputs["positions"], np.int32)
    w_qa = np.asarray(inputs["w_qa"], np.float32)
    q_ln = np.asarray(inputs["q_a_ln_w"], np.float32)
    w_qb = np.asarray(inputs["w_qb"], np.float32)
    w_kva = np.asarray(inputs["w_kva"], np.float32)
    kv_ln = np.asarray(inputs["kv_a_ln_w"], np.float32)
    w_kvb = np.asarray(inputs["w_kvb"], np.float32)
    w_o = np.asarray(inputs["w_o"], np.float32)

    # a-projections, pre-tiled: [p, m, k, col]
    wqa_b = np.ascontiguousarray(
        w_qa.reshape(NKH, P, NKQ, P).transpose(1, 2, 0, 3)
    ).astype(bf)

    def rot_cols(A):
        return np.concatenate([-A[:, DR // 2 :], A[:, : DR // 2]], axis=1)

    kva_lat = w_kva[:, :KVL]
    kva_rope = w_kva[:, KVL:]                      # [2048, 64]
    kva_ropeB = rot_cols(kva_rope)
    wkva_aug = np.concatenate(
        [kva_lat, kva_rope, kva_rope, kva_ropeB, kva_ropeB], axis=1
    )  # [2048, 512+128+128]
    wkva_b = np.ascontiguousarray(
        wkva_aug.reshape(NKH, P, NKV + 2, P).transpose(1, 2, 0, 3)
    ).astype(bf)

    # rope tables (dup-row structure)
    inv_freq = (
        1.0 / (10000.0 ** (np.arange(0, DR, 2, dtype=np.float32) / DR))
    ).astype(np.float32)
    freqs = pos.astype(np.float32)[:, None] * inv_freq[None, :]
    emb = np.concatenate([freqs, freqs], axis=-1)  # [T, 64]
    cosT = np.ascontiguousarray(np.cos(emb).T.astype(np.float32))  # [64, T]
    sinT = np.ascontiguousarray(np.sin(emb).T.astype(np.float32))
    cos2 = np.ascontiguousarray(np.concatenate([cosT, cosT], axis=0))  # [128, T]
    sin2 = np.ascontiguousarray(np.concatenate([sinT, sinT], axis=0))

    # q b-projection, ALL head-pairs, ln folded: [p, pair, mo, k, col]
    w_qb_f = (w_qb * q_ln[:, None]).reshape(QL, H, DQK)
    blocks = []
    for p_ in range(NP):
        h0, h1 = 2 * p_, 2 * p_ + 1
        ropeA = np.concatenate(
            [w_qb_f[:, h0, DN:], w_qb_f[:, h1, DN:]], axis=1
        )  # [QL, 128]
        ropeB = np.concatenate(
            [rot_cols(w_qb_f[:, h0, DN:]), rot_cols(w_qb_f[:, h1, DN:])], axis=1
        )
        blocks.append(
            np.stack(
                [w_qb_f[:, h0, :DN], w_qb_f[:, h1, :DN], ropeA, ropeB], axis=0
            )  # [4, QL, 128]
        )
    wqb_all = np.stack(blocks, axis=0)  # [NP, 4, QL, 128]
    wqb_aug = np.ascontiguousarray(
        wqb_all.reshape(NP, 4, NKQ, P, P).transpose(3, 0, 1, 2, 4)
    ).astype(bf)  # [p, pair, mo, k, col]

    w_kvb_f = (w_kvb * kv_ln[:, None]).reshape(KVL, H, DN + DV)
    w_o_r = w_o.reshape(H, DV, HID)
    trimask = np.triu(np.ones((P, P), dtype=np.float32)).astype(bf)

    cosf = cos2.astype(bf)
    sinf = sin2.astype(bf)

    per_core = []
    for i in range(NC_):
        hh = [HLOC * i + x for x in range(HLOC)]
        t0 = i * TC
        hT = np.ascontiguousarray(
            hs[t0 : t0 + TC].reshape(TC, NKH, P).transpose(2, 1, 0)
        ).astype(bf)
        wkvbk = np.ascontiguousarray(
            np.concatenate([w_kvb_f[:, h, :DN] for h in hh], axis=1)
            .reshape(NKV, P, HLOC * DN)
            .transpose(1, 0, 2)
        ).astype(bf)
        wkvbv = np.ascontiguousarray(
            np.concatenate([w_kvb_f[:, h, DN:] for h in hh], axis=1)
            .reshape(NKV, P, HLOC * DV)
            .transpose(1, 0, 2)
        ).astype(bf)
        wo_i = np.ascontiguousarray(
            np.stack([w_o_r[h] for h in hh], axis=0).transpose(1, 0, 2)
        ).astype(bf)  # [p, h, HID]
        per_core.append(
            dict(
                hT=hT,
                wqa=wqa_b,
                wkva=wkva_b,
                wqb=wqb_aug,
                wkvbk=wkvbk,
                wkvbv=wkvbv,
                wo=wo_i,
                cosl=np.ascontiguousarray(cos2[:, t0 : t0 + TC]),
                sinl=np.ascontiguousarray(sin2[:, t0 : t0 + TC]),
                cosf=cosf,
                sinf=sinf,
                trimask=trimask,
            )
        )
    return per_core


def kernel(**inputs):
    global LAST_RESULTS
    from concourse.bass_utils import run_bass_kernel_spmd

    if "nc" not in _CACHE:
        _CACHE["nc"] = _build_program()
    nc = _CACHE["nc"]

    in_maps = _host_prep(inputs)
    res = run_bass_kernel_spmd(nc, in_maps, core_ids=list(range(NC_)))
    LAST_RESULTS = res
    out = np.zeros((T, HID), dtype=np.float32)
    for r in res.results:
        out += np.asarray(r["y"], dtype=np.float32)
    return out


# revision 11
# speedup vs baseline: 1.4443x; 1.1053x over previous
"""DeepseekV2 MLA attention (T=2048, H=16) on 8 trn2 cores.

Sharding v2 (collective-based, no replicated stage-1):
- Stage 1 (the big a-projections) is sequence-sharded: each core computes
  q_c / kv latent / roped k_pe for its own 256 tokens only.
- The q up-projection stays sequence-sharded (own tokens, ALL 16 heads,
  rms scale and neox rope folded in pre-exchange); an AllToAll then
  redistributes q by head-pair so each core ends with its 2 heads over
  the full sequence (3 tiles per pair: nope h0, nope h1, roped q_pe).
- The kv latent (+ roped k_pe) is AllGather'd (tiny: the point of MLA),
  and each core up-projects K/V for its 2 heads locally.
- A dummy collective at t=0 absorbs the one-time ~56us comm-init cost
  while stage 1 runs on the PE.
- Attention is head-sharded like the baseline: S^T = K^T q blocks, no
  row-max (logits ~N(0,1)), denominator via ones-matmul, normalization
  after P@V, causal diag via 0/1 tri mask post-exp, o_proj partials
  summed on the host.
- dtypes: stage-1/up-proj matmuls bf16 at free-dim 256; attention and
  o_proj keep the MOVING operand f32r (measured: bf16 moving at free-512
  runs 356ns vs 252ns f32r) with bf16 stationary operands.
- DMA triggers are spread by engine so a semaphore-gated trigger can
  never head-of-line-block a weight stream: sync = weight streams,
  scalar = persists + payload writes + collective readbacks, gpsimd =
  collectives only.
"""

import numpy as np

T = 2048
HID = 2048
H = 16
NC_ = 8
HLOC = H // NC_          # 2 heads per core
NP = H // HLOC           # 8 head-pairs
QL = 1536                # q lora
KVL = 512                # kv lora
DN = 128                 # nope dim
DR = 64                  # rope dim
DQK = DN + DR            # 192
DV = 128
EPS = 1e-6
SCALE = float(DQK) ** -0.5
P = 128
TC = T // NC_            # 256 tokens per core (stage-1 shard)
NKQ = QL // P            # 12
NKV = KVL // P           # 4
NKH = HID // P           # 16
QC = 512                 # attention q-chunk
NQC = T // QC
NKB = T // P             # key blocks

_CACHE = {}
LAST_RESULTS = None


def _split_multi_waits(nc, mybir):
    """Walrus embeds at most one sem/event wait per TPB instruction; hoist
    extra waits onto preceding same-engine NoOps (queue FIFO keeps order)."""
    n = 0
    for f in nc.m.functions:
        for bb in f.blocks:
            new = []
            for inst in bb.instructions:
                si = getattr(inst, "sync_info", None)
                if si is not None and len(si.on_wait) > 1:
                    waits = list(si.on_wait)
                    for i, wv in enumerate(waits[:-1]):
                        noop = mybir.InstNoOp(
                            name=f"{inst.name}-wsplit{i}",
                            engine=inst.engine,
                            ins=[],
                            outs=[],
                        )
                        noop.bass_nofuse = True
                        noop.sync_info = mybir.SyncInfo(on_wait=[wv], on_update=[])
                        new.append(noop)
                    inst.sync_info = mybir.SyncInfo(
                        on_wait=[waits[-1]], on_update=list(si.on_update)
                    )
                    n += 1
                new.append(inst)
            bb.instructions = new
    return n


def _build_program():
    import concourse.bass as bass
    import concourse.tile as tile
    from concourse import mybir

    f32 = mybir.dt.float32
    bf16 = mybir.dt.bfloat16
    f32r = mybir.dt.float32r
    AF = mybir.ActivationFunctionType
    GRP = [list(range(NC_))]

    nc = bass.Bass(num_devices=NC_)

    def r32(ap):
        return ap.bitcast(f32r)

    # ---- dram parameters (per-core values supplied by the host) ----
    hT_d = nc.declare_dram_parameter("hT", [P, NKH, TC], bf16, isOutput=False)
    wqa_d = nc.declare_dram_parameter("wqa", [P, NKQ, NKH, P], bf16, isOutput=False)
    # latent 512 | ropeA dup 128 | ropeB dup 128
    wkva_d = nc.declare_dram_parameter("wkva", [P, NKV + 2, NKH, P], bf16, isOutput=False)
    # full q b-projection for ALL head-pairs: [p, pair, mo, k, col]
    # mo: 0/1 = nope h0/h1, 2 = ropeA dup, 3 = ropeB dup  (ln folded)
    wqb_d = nc.declare_dram_parameter("wqb", [P, NP, 4, NKQ, P], bf16, isOutput=False)
    wkvbk_d = nc.declare_dram_parameter("wkvbk", [P, NKV, HLOC * DN], bf16, isOutput=False)
    wkvbv_d = nc.declare_dram_parameter("wkvbv", [P, NKV, HLOC * DV], bf16, isOutput=False)
    wo_d = nc.declare_dram_parameter("wo", [P, HLOC, HID], f32r, isOutput=False)
    cosl_d = nc.declare_dram_parameter("cosl", [P, TC], f32, isOutput=False)
    sinl_d = nc.declare_dram_parameter("sinl", [P, TC], f32, isOutput=False)
    trimask_d = nc.declare_dram_parameter("trimask", [P, P], f32, isOutput=False)
    y_d = nc.declare_dram_parameter("y", [T, HID], f32, isOutput=True)

    # ---- dram bounce buffers for the collectives ----
    d_in = nc.dram_tensor("d_in", [P, 1], f32)
    d_out = nc.dram_tensor("d_out", [NC_, P, 1], f32, addr_space="Shared")
    # kv payload: 4 latent tiles + 1 roped kpe (dup) tile, [p, m, t]
    kv_in = nc.dram_tensor("kv_in", [P, NKV + 1, TC], bf16)
    kv_out = nc.dram_tensor(
        "kv_out", [NC_, P, NKV + 1, TC], bf16, addr_space="Shared"
    )
    # q payload: per dst head-pair [p, mo(3), t]: nope h0 | nope h1 | roped qpe
    q_in = nc.dram_tensor("q_in", [NP, P, 3, TC], bf16)
    q_out = nc.dram_tensor("q_out", [NC_, P, 3, TC], bf16)

    with tile.TileContext(nc) as tc, nc.allow_low_precision(
        reason="bf16/f32r matmul operands are intentional"
    ):
        # dummy collective: warms up the comm channel during stage-1
        nc.gpsimd.collective_compute(
            "AllGather",
            mybir.AluOpType.bypass,
            replica_groups=GRP,
            ins=[d_in[:, :].opt()],
            outs=[d_out[:, :, :].opt()],
        )

        with tc.tile_pool(name="persist", bufs=1) as pp:
            # persistent loads on the scalar queue (no waits -> no blocking)
            wkvbk_sb = pp.tile([P, NKV, HLOC * DN], bf16, name="wkvbk")
            nc.scalar.dma_start(out=wkvbk_sb, in_=wkvbk_d[:, :, :])
            wkvbv_sb = pp.tile([P, NKV, HLOC * DV], bf16, name="wkvbv")
            nc.scalar.dma_start(out=wkvbv_sb, in_=wkvbv_d[:, :, :])
            cosl_sb = pp.tile([P, TC], f32, name="cosl")
            nc.scalar.dma_start(out=cosl_sb, in_=cosl_d[:, :])
            sinl_sb = pp.tile([P, TC], f32, name="sinl")
            nc.scalar.dma_start(out=sinl_sb, in_=sinl_d[:, :])
            trimask_sb = pp.tile([P, P], f32, name="trimask")
            nc.scalar.dma_start(out=trimask_sb, in_=trimask_d[:, :])
            wo_sb = pp.tile([P, HLOC, HID], f32r, name="wo")
            nc.scalar.dma_start(out=wo_sb, in_=wo_d[:, :, :])

            ones_f = pp.tile([P, P], f32, name="ones_f")
            nc.vector.memset(ones_f, 1.0)
            ones_sb = pp.tile([P, 1], f32r, name="ones")
            nc.vector.tensor_copy(ones_sb, ones_f[:, 0:1])
            col_ones = pp.tile([1, P], f32r, name="col_ones")
            nc.vector.tensor_copy(col_ones, ones_f[0:1, :])
            zmask = pp.tile([P, HLOC], f32, name="zmask")
            nc.vector.memset(zmask[0:DR, 0:1], 1.0)
            nc.vector.memset(zmask[DR:P, 0:1], 0.0)
            nc.vector.memset(zmask[0:DR, 1:2], 0.0)
            nc.vector.memset(zmask[DR:P, 1:2], 1.0)
            eps_sb = pp.tile([1, 1], f32, name="eps")
            nc.vector.memset(eps_sb, EPS)

            pay_kv = pp.tile([P, NKV + 1, TC], bf16, name="paykv")
            # pay_q doubles as the post-AllToAll bf16 staging buffer
            pay_q = pp.tile([P, NP * 3, TC], bf16, name="payq")
            rq_b = pp.tile([P, TC], f32, name="rqb")
            rkv_b = pp.tile([P, TC], f32, name="rkvb")

            # attention matmul operands in f32r: measured bf16 moving
            # operands at free-dim 512 run 1.4x slower than f32r
            qTn = [pp.tile([P, T], f32r, name=f"qTn{h}") for h in range(HLOC)]
            qpeT2 = pp.tile([P, T], f32r, name="qpeT2")
            KT = [pp.tile([P, T], f32r, name=f"KT{h}") for h in range(HLOC)]
            kpe_raw = pp.tile([P, T], bf16, name="kperaw")
            kpe2 = [pp.tile([P, T], f32r, name=f"kpe2{h}") for h in range(HLOC)]
            kvn_sb = pp.tile([P, NKV, T], bf16, name="kvn")
            V_sb = [pp.tile([P, HLOC * DV], f32r, name=f"v{i}") for i in range(NKB)]

            # ---------------- Stage A: sharded projections ----------------
            with (
                tc.tile_pool(name="astream", bufs=2) as sp_,
                tc.tile_pool(name="aqbstream", bufs=3) as qbp,
                tc.tile_pool(name="asmall", bufs=1) as smp,
                tc.tile_pool(name="aps", bufs=2, space="PSUM") as s1ps,
                tc.tile_pool(name="arope", bufs=1, space="PSUM") as rps,
                tc.tile_pool(name="ssqps", bufs=1, space="PSUM") as ssqps,
                tc.tile_pool(name="upps", bufs=2, space="PSUM") as upps,
            ):
                h_sb = sp_.tile([P, NKH, TC], bf16, name="hchunk", bufs=1)
                nc.sync.dma_start(out=h_sb, in_=hT_d[:, :, :])
                qc_sb = sp_.tile([P, NKQ, TC], bf16, name="qc", bufs=1)
                ssq_kv = ssqps.tile([1, TC], f32, name="ssqkv")
                ssq_q = ssqps.tile([1, TC], f32, name="ssqq")

                # --- kv path first (its payload gates CC#1) ---
                rope_ps = []
                for m in range(NKV + 2):
                    wk_sb = sp_.tile([P, NKH, P], bf16, name="wstream")
                    nc.sync.dma_start(out=wk_sb, in_=wkva_d[:, m, :, :])
                    if m < NKV:
                        ps = s1ps.tile([P, TC], f32, name="s1")
                    else:
                        ps = rps.tile([P, TC], f32, name=f"rope{m - NKV}")
                    for k in range(NKH):
                        nc.tensor.matmul(
                            ps,
                            lhsT=wk_sb[:, k, :],
                            rhs=h_sb[:, k, :],
                            start=(k == 0),
                            stop=(k == NKH - 1),
                        )
                    if m < NKV:
                        nc.vector.tensor_copy(pay_kv[:, m, :], ps)
                        sq = smp.tile([P, TC], f32r, name="sq", bufs=1)
                        nc.scalar.square(sq, ps)
                        nc.tensor.matmul(
                            ssq_kv,
                            lhsT=ones_sb,
                            rhs=sq,
                            start=(m == 0),
                            stop=(m == NKV - 1),
                        )
                    else:
                        rope_ps.append(ps)

                # rkv scale + broadcast
                rkv = smp.tile([1, TC], f32r, name="rkv")
                nc.scalar.activation(
                    rkv, ssq_kv, func=AF.Sqrt, bias=eps_sb, scale=1.0 / KVL
                )
                nc.vector.reciprocal(rkv, rkv)
                rkvb_ps = upps.tile([P, TC], f32, name="up")
                nc.tensor.matmul(rkvb_ps, lhsT=col_ones, rhs=rkv, start=True, stop=True)
                nc.vector.tensor_copy(rkv_b, rkvb_ps)
                # roped k_pe (dup rows), then normalize latent payload
                t1 = smp.tile([P, TC], f32, name="ropet1")
                t2 = smp.tile([P, TC], f32, name="ropet2")
                nc.vector.tensor_mul(t1, rope_ps[0], cosl_sb)
                nc.vector.tensor_mul(t2, rope_ps[1], sinl_sb)
                nc.vector.tensor_add(pay_kv[:, NKV, :], t1, t2)
                for m in range(NKV):
                    nc.vector.tensor_mul(pay_kv[:, m, :], pay_kv[:, m, :], rkv_b)
                nc.scalar.dma_start(out=kv_in[:, :, :], in_=pay_kv)
                nc.gpsimd.collective_compute(
                    "AllGather",
                    mybir.AluOpType.bypass,
                    replica_groups=GRP,
                    ins=[kv_in[:, :, :].opt()],
                    outs=[kv_out[:, :, :, :].opt()],
                )

                # --- q path stage-1 ---
                for m in range(NKQ):
                    wq_sb = sp_.tile([P, NKH, P], bf16, name="wstream")
                    nc.sync.dma_start(out=wq_sb, in_=wqa_d[:, m, :, :])
                    ps = s1ps.tile([P, TC], f32, name="s1")
                    for k in range(NKH):
                        nc.tensor.matmul(
                            ps,
                            lhsT=wq_sb[:, k, :],
                            rhs=h_sb[:, k, :],
                            start=(k == 0),
                            stop=(k == NKH - 1),
                        )
                    nc.vector.tensor_copy(qc_sb[:, m, :], ps)
                    sq = smp.tile([P, TC], f32r, name="sq", bufs=1)
                    nc.scalar.square(sq, ps)
                    nc.tensor.matmul(
                        ssq_q,
                        lhsT=ones_sb,
                        rhs=sq,
                        start=(m == 0),
                        stop=(m == NKQ - 1),
                    )
                rq = smp.tile([1, TC], f32r, name="rq")
                nc.scalar.activation(
                    rq, ssq_q, func=AF.Sqrt, bias=eps_sb, scale=1.0 / QL
                )
                nc.vector.reciprocal(rq, rq)
                rqb_ps = upps.tile([P, TC], f32, name="up")
                nc.tensor.matmul(rqb_ps, lhsT=col_ones, rhs=rq, start=True, stop=True)
                nc.vector.tensor_copy(rq_b, rqb_ps)

                # --- q up-projection: own tokens, ALL head-pairs ---
                for p_ in range(NP):
                    ups = []
                    for mo in range(4):
                        wqbs = qbp.tile([P, NKQ, P], bf16, name="wqbs")
                        nc.sync.dma_start(out=wqbs, in_=wqb_d[:, p_, mo, :, :])
                        ps = upps.tile([P, TC], f32, name="up")
                        for k in range(NKQ):
                            nc.tensor.matmul(
                                ps,
                                lhsT=wqbs[:, k, :],
                                rhs=qc_sb[:, k, :],
                                start=(k == 0),
                                stop=(k == NKQ - 1),
                            )
                        if mo < 2:
                            nc.vector.tensor_mul(
                                pay_q[:, p_ * 3 + mo, :], ps, rq_b
                            )
                        else:
                            ups.append(ps)
                    # neox rope on the decoupled dims, folded pre-exchange
                    t5 = smp.tile([P, TC], f32, name="qropet1")
                    t6 = smp.tile([P, TC], f32, name="qropet2")
                    nc.vector.tensor_mul(t5, ups[0], cosl_sb)
                    nc.vector.tensor_mul(t6, ups[1], sinl_sb)
                    nc.vector.tensor_add(t5, t5, t6)
                    nc.vector.tensor_mul(pay_q[:, p_ * 3 + 2, :], t5, rq_b)
                    nc.scalar.dma_start(
                        out=q_in[p_, :, :, :],
                        in_=pay_q[:, p_ * 3 : p_ * 3 + 3, :],
                    )
                nc.gpsimd.collective_compute(
                    "AllToAll",
                    mybir.AluOpType.bypass,
                    replica_groups=GRP,
                    ins=[q_in[:, :, :, :].opt()],
                    outs=[q_out[:, :, :, :].opt()],
                )

            # ---------------- Stage B: gather-side compute ----------------
            with (
                tc.tile_pool(name="bpt", bufs=4) as ptp,
                tc.tile_pool(name="bsmall", bufs=3) as bsm,
                tc.tile_pool(name="sps", bufs=2, space="PSUM") as spsp,
                tc.tile_pool(name="otps", bufs=2, space="PSUM") as otpsp,
                tc.tile_pool(name="lps", bufs=2, space="PSUM") as lpsp,
            ):
                # kv readback + K/V up-projection for own heads
                for r in range(NC_):
                    nc.scalar.dma_start(
                        out=kvn_sb[:, :, r * TC : (r + 1) * TC],
                        in_=kv_out[r, :, 0:NKV, :],
                    )
                    nc.scalar.dma_start(
                        out=kpe_raw[:, r * TC : (r + 1) * TC],
                        in_=kv_out[r, :, NKV, :],
                    )
                for h in range(HLOC):
                    nc.vector.tensor_scalar_mul(
                        kpe2[h], kpe_raw, zmask[:, h : h + 1]
                    )
                for h in range(HLOC):
                    for j in range(T // QC):
                        ps = otpsp.tile([P, QC], f32, name="otps")
                        for k in range(NKV):
                            nc.tensor.matmul(
                                ps,
                                lhsT=wkvbk_sb[:, k, h * P : (h + 1) * P],
                                rhs=kvn_sb[:, k, j * QC : (j + 1) * QC],
                                start=(k == 0),
                                stop=(k == NKV - 1),
                            )
                        nc.vector.tensor_copy(KT[h][:, j * QC : (j + 1) * QC], ps)
                for tt in range(NKB):
                    ps = otpsp.tile([P, QC], f32, name="otps")[:, : HLOC * DV]
                    for k in range(NKV):
                        nc.tensor.matmul(
                            ps,
                            lhsT=kvn_sb[:, k, tt * P : (tt + 1) * P],
                            rhs=wkvbv_sb[:, k, :],
                            start=(k == 0),
                            stop=(k == NKV - 1),
                        )
                    nc.vector.tensor_copy(V_sb[tt], ps)

                # q readback into pay_q (dead after the AllToAll consumed
                # q_in) then widen the moving operands to f32
                for r in range(NC_):
                    nc.scalar.dma_start(
                        out=pay_q[:, r, :], in_=q_out[r, :, 0, :]
                    )
                    nc.scalar.dma_start(
                        out=pay_q[:, NC_ + r, :], in_=q_out[r, :, 1, :]
                    )
                    nc.scalar.dma_start(
                        out=pay_q[:, 2 * NC_ + r, :], in_=q_out[r, :, 2, :]
                    )
                for h in range(HLOC):
                    nc.vector.tensor_copy(
                        qTn[h], pay_q[:, h * NC_ : (h + 1) * NC_, :]
                    )
                nc.vector.tensor_copy(qpeT2, pay_q[:, 2 * NC_ : 3 * NC_, :])

                # ---------------- attention ----------------
                OT_sb = [
                    [ptp.tile([P, QC], f32r, name=f"ot{h}_{j}", bufs=1) for j in range(NQC)]
                    for h in range(HLOC)
                ]

                def flush_norm(pend):
                    p_ot, p_l, p_h, p_j = pend
                    recl = bsm.tile([1, QC], f32r, name="recl")
                    nc.vector.reciprocal(recl, p_l)
                    lb_ps = spsp.tile([P, 2 * QC], f32, name="sps2")[:, :QC]
                    nc.tensor.matmul(lb_ps, lhsT=col_ones, rhs=recl, start=True, stop=True)
                    lb = bsm.tile([P, QC], f32, name="lb")
                    nc.scalar.copy(lb, lb_ps)
                    nc.vector.tensor_mul(OT_sb[p_h][p_j], p_ot, lb)

                pend = None
                for h in range(HLOC):
                    for j in range(NQC):
                        ot_ps = otpsp.tile([P, QC], f32, name="otps")
                        l_ps = lpsp.tile([1, QC], f32, name="lps")
                        nkb = 4 * (j + 1)
                        qcol0 = j * QC
                        for kp in range(0, nkb, 2):
                            # two k-blocks share one PSUM pair and ONE wide exp
                            s2 = spsp.tile([P, 2 * QC], f32, name="sps2")
                            for u in range(2):
                                ki = kp + u
                                nc.tensor.matmul(
                                    s2[:, u * QC : (u + 1) * QC],
                                    lhsT=KT[h][:, ki * P : (ki + 1) * P],
                                    rhs=qTn[h][:, qcol0 : qcol0 + QC],
                                    start=True,
                                    stop=False,
                                )
                                nc.tensor.matmul(
                                    s2[:, u * QC : (u + 1) * QC],
                                    lhsT=kpe2[h][:, ki * P : (ki + 1) * P],
                                    rhs=qpeT2[:, qcol0 : qcol0 + QC],
                                    start=False,
                                    stop=True,
                                )
                            pt = ptp.tile([P, 2 * QC], f32r, name="pt")
                            nc.scalar.activation(pt, s2, func=AF.Exp, scale=SCALE)
                            for u in range(2):
                                ki = kp + u
                                diag = (ki // 4 == j)
                                cs = (ki % 4) * P if diag else 0
                                if diag:
                                    nc.gpsimd.tensor_mul(
                                        pt[:, u * QC + cs : u * QC + cs + P],
                                        pt[:, u * QC + cs : u * QC + cs + P],
                                        trimask_sb,
                                    )
                                nc.tensor.matmul(
                                    ot_ps[:, cs:],
                                    lhsT=V_sb[ki][:, h * DV : (h + 1) * DV],
                                    rhs=pt[:, u * QC + cs : (u + 1) * QC],
                                    start=(ki == 0),
                                    stop=(ki == nkb - 1),
                                )
                                nc.tensor.matmul(
                                    l_ps[:, cs:],
                                    lhsT=ones_sb,
                                    rhs=pt[:, u * QC + cs : (u + 1) * QC],
                                    start=(ki == 0),
                                    stop=(ki == nkb - 1),
                                )
                            if kp == 2 and pend is not None:
                                flush_norm(pend)
                                pend = None
                        pend = (ot_ps, l_ps, h, j)
                flush_norm(pend)

                # ---------------- o_proj ----------------
                for tt in range(T // P):
                    j, sub = tt // 4, (tt % 4) * P
                    for n in range(HID // QC):
                        y_ps = spsp.tile([P, 2 * QC], f32, name="sps2")[:, :QC]
                        for h in range(HLOC):
                            nc.tensor.matmul(
                                y_ps,
                                lhsT=OT_sb[h][j][:, sub : sub + P],
                                rhs=wo_sb[:, h, n * QC : (n + 1) * QC],
                                start=(h == 0),
                                stop=(h == HLOC - 1),
                            )
                        y_sb = ptp.tile([P, QC], f32, name="ysb")
                        nc.vector.tensor_copy(y_sb, y_ps)
                        nc.sync.dma_start(
                            out=y_d[tt * P : (tt + 1) * P, n * QC : (n + 1) * QC],
                            in_=y_sb,
                        )
    _split_multi_waits(nc, mybir)
    return nc


def _host_prep(inputs):
    import ml_dtypes

    bf = ml_dtypes.bfloat16
    hs = np.ascontiguousarray(np.asarray(inputs["hidden_states"], np.float32))
    pos = np.asarray(inputs["positions"], np.int32)
    w_qa = np.asarray(inputs["w_qa"], np.float32)
    q_ln = np.asarray(inputs["q_a_ln_w"], np.float32)
    w_qb = np.asarray(inputs["w_qb"], np.float32)
    w_kva = np.asarray(inputs["w_kva"], np.float32)
    kv_ln = np.asarray(inputs["kv_a_ln_w"], np.float32)
    w_kvb = np.asarray(inputs["w_kvb"], np.float32)
    w_o = np.asarray(inputs["w_o"], np.float32)

    # a-projections, pre-tiled: [p, m, k, col]
    wqa_b = np.ascontiguousarray(
        w_qa.reshape(NKH, P, NKQ, P).transpose(1, 2, 0, 3)
    ).astype(bf)

    def rot_cols(A):
        return np.concatenate([-A[:, DR // 2 :], A[:, : DR // 2]], axis=1)

    kva_lat = w_kva[:, :KVL]
    kva_rope = w_kva[:, KVL:]                      # [2048, 64]
    kva_ropeB = rot_cols(kva_rope)
    wkva_aug = np.concatenate(
        [kva_lat, kva_rope, kva_rope, kva_ropeB, kva_ropeB], axis=1
    )  # [2048, 512+128+128]
    wkva_b = np.ascontiguousarray(
        wkva_aug.reshape(NKH, P, NKV + 2, P).transpose(1, 2, 0, 3)
    ).astype(bf)

    # rope tables (dup-row structure)
    inv_freq = (
        1.0 / (10000.0 ** (np.arange(0, DR, 2, dtype=np.float32) / DR))
    ).astype(np.float32)
    freqs = pos.astype(np.float32)[:, None] * inv_freq[None, :]
    emb = np.concatenate([freqs, freqs], axis=-1)  # [T, 64]
    cosT = np.ascontiguousarray(np.cos(emb).T.astype(np.float32))  # [64, T]
    sinT = np.ascontiguousarray(np.sin(emb).T.astype(np.float32))
    cos2 = np.ascontiguousarray(np.concatenate([cosT, cosT], axis=0))  # [128, T]
    sin2 = np.ascontiguousarray(np.concatenate([sinT, sinT], axis=0))

    # q b-projection, ALL head-pairs, ln folded: [p, pair, mo, k, col]
    w_qb_f = (w_qb * q_ln[:, None]).reshape(QL, H, DQK)
    blocks = []
    for p_ in range(NP):
        h0, h1 = 2 * p_, 2 * p_ + 1
        ropeA = np.concatenate(
            [w_qb_f[:, h0, DN:], w_qb_f[:, h1, DN:]], axis=1
        )  # [QL, 128]
        ropeB = np.concatenate(
            [rot_cols(w_qb_f[:, h0, DN:]), rot_cols(w_qb_f[:, h1, DN:])], axis=1
        )
        blocks.append(
            np.stack(
                [w_qb_f[:, h0, :DN], w_qb_f[:, h1, :DN], ropeA, ropeB], axis=0
            )  # [4, QL, 128]
        )
    wqb_all = np.stack(blocks, axis=0)  # [NP, 4, QL, 128]
    wqb_aug = np.ascontiguousarray(
        wqb_all.reshape(NP, 4, NKQ, P, P).transpose(3, 0, 1, 2, 4)
    ).astype(bf)  # [p, pair, mo, k, col]

    w_kvb_f = (w_kvb * kv_ln[:, None]).reshape(KVL, H, DN + DV)
    w_o_r = w_o.reshape(H, DV, HID)
    trimask = np.triu(np.ones((P, P), dtype=np.float32))

    per_core = []
    for i in range(NC_):
        hh = [HLOC * i + x for x in range(HLOC)]
        t0 = i * TC
        hT = np.ascontiguousarray(
            hs[t0 : t0 + TC].reshape(TC, NKH, P).transpose(2, 1, 0)
        ).astype(bf)
        wkvbk = np.ascontiguousarray(
            np.concatenate([w_kvb_f[:, h, :DN] for h in hh], axis=1)
            .reshape(NKV, P, HLOC * DN)
            .transpose(1, 0, 2)
        ).astype(bf)
        wkvbv = np.ascontiguousarray(
            np.concatenate([w_kvb_f[:, h, DN:] for h in hh], axis=1)
            .reshape(NKV, P, HLOC * DV)
            .transpose(1, 0, 2)
        ).astype(bf)
        wo_i = np.ascontiguousarray(
            np.stack([w_o_r[h] for h in hh], axis=0).transpose(1, 0, 2)
        )  # [p, h, HID] f32
        per_core.append(
            dict(
                hT=hT,
                wqa=wqa_b,
                wkva=wkva_b,
                wqb=wqb_aug,
                wkvbk=wkvbk,
                wkvbv=wkvbv,
                wo=wo_i,
                cosl=np.ascontiguousarray(cos2[:, t0 : t0 + TC]),
                sinl=np.ascontiguousarray(sin2[:, t0 : t0 + TC]),
                trimask=trimask,
            )
        )
    return per_core


def kernel(**inputs):
    global LAST_RESULTS
    from concourse.bass_utils import run_bass_kernel_spmd

    if "nc" not in _CACHE:
        _CACHE["nc"] = _build_program()
    nc = _CACHE["nc"]

    in_maps = _host_prep(inputs)
    res = run_bass_kernel_spmd(nc, in_maps, core_ids=list(range(NC_)))
    LAST_RESULTS = res
    out = np.zeros((T, HID), dtype=np.float32)
    for r in res.results:
        out += np.asarray(r["y"], dtype=np.float32)
    return out


# revision 13
# speedup vs baseline: 1.4763x; 1.0221x over previous
"""DeepseekV2 MLA attention (T=2048, H=16) on 8 trn2 cores.

Sharding v2 (collective-based, no replicated stage-1):
- Stage 1 (the big a-projections) is sequence-sharded: each core computes
  q_c / kv latent / roped k_pe for its own 256 tokens only.
- The q up-projection stays sequence-sharded (own tokens, ALL 16 heads,
  rms scale and neox rope folded in pre-exchange); an AllToAll then
  redistributes q by head-pair so each core ends with its 2 heads over
  the full sequence (3 tiles per pair: nope h0, nope h1, roped q_pe).
- The kv latent (+ roped k_pe) is AllGather'd (tiny: the point of MLA),
  and each core up-projects K/V for its 2 heads locally.
- A dummy collective at t=0 absorbs the one-time ~56us comm-init cost
  while stage 1 runs on the PE.
- Attention is head-sharded like the baseline: S^T = K^T q blocks, no
  row-max (logits ~N(0,1)), denominator via ones-matmul, normalization
  after P@V, causal diag via 0/1 tri mask post-exp, o_proj partials
  summed on the host.
- dtypes: stage-1/up-proj matmuls bf16 at free-dim 256; attention and
  o_proj keep the MOVING operand f32r (measured: bf16 moving at free-512
  runs 356ns vs 252ns f32r) with bf16 stationary operands.
- DMA triggers are spread by engine so a semaphore-gated trigger can
  never head-of-line-block a weight stream: sync = weight streams,
  scalar = persists + payload writes + collective readbacks, gpsimd =
  collectives only.
"""

import numpy as np

T = 2048
HID = 2048
H = 16
NC_ = 8
HLOC = H // NC_          # 2 heads per core
NP = H // HLOC           # 8 head-pairs
QL = 1536                # q lora
KVL = 512                # kv lora
DN = 128                 # nope dim
DR = 64                  # rope dim
DQK = DN + DR            # 192
DV = 128
EPS = 1e-6
SCALE = float(DQK) ** -0.5
P = 128
TC = T // NC_            # 256 tokens per core (stage-1 shard)
NKQ = QL // P            # 12
NKV = KVL // P           # 4
NKH = HID // P           # 16
QC = 512                 # attention q-chunk
NQC = T // QC
NKB = T // P             # key blocks

_CACHE = {}
LAST_RESULTS = None


def _split_multi_waits(nc, mybir):
    """Walrus embeds at most one sem/event wait per TPB instruction; hoist
    extra waits onto preceding same-engine NoOps (queue FIFO keeps order)."""
    n = 0
    for f in nc.m.functions:
        for bb in f.blocks:
            new = []
            for inst in bb.instructions:
                si = getattr(inst, "sync_info", None)
                if si is not None and len(si.on_wait) > 1:
                    waits = list(si.on_wait)
                    for i, wv in enumerate(waits[:-1]):
                        noop = mybir.InstNoOp(
                            name=f"{inst.name}-wsplit{i}",
                            engine=inst.engine,
                            ins=[],
                            outs=[],
                        )
                        noop.bass_nofuse = True
                        noop.sync_info = mybir.SyncInfo(on_wait=[wv], on_update=[])
                        new.append(noop)
                    inst.sync_info = mybir.SyncInfo(
                        on_wait=[waits[-1]], on_update=list(si.on_update)
                    )
                    n += 1
                new.append(inst)
            bb.instructions = new
    return n


def _build_program():
    import concourse.bass as bass
    import concourse.tile as tile
    from concourse import mybir

    f32 = mybir.dt.float32
    bf16 = mybir.dt.bfloat16
    f32r = mybir.dt.float32r
    AF = mybir.ActivationFunctionType
    GRP = [list(range(NC_))]

    nc = bass.Bass(num_devices=NC_)

    def r32(ap):
        return ap.bitcast(f32r)

    # ---- dram parameters (per-core values supplied by the host) ----
    hT_d = nc.declare_dram_parameter("hT", [P, NKH, TC], bf16, isOutput=False)
    wqa_d = nc.declare_dram_parameter("wqa", [P, NKQ, NKH, P], bf16, isOutput=False)
    # latent 512 | ropeA dup 128 | ropeB dup 128
    wkva_d = nc.declare_dram_parameter("wkva", [P, NKV + 2, NKH, P], bf16, isOutput=False)
    # full q b-projection for ALL head-pairs: [p, pair, mo, k, col]
    # mo: 0/1 = nope h0/h1, 2 = ropeA dup, 3 = ropeB dup  (ln folded)
    wqb_d = nc.declare_dram_parameter("wqb", [P, NP, 4, NKQ, P], bf16, isOutput=False)
    wkvbk_d = nc.declare_dram_parameter("wkvbk", [P, NKV, HLOC * DN], bf16, isOutput=False)
    wkvbv_d = nc.declare_dram_parameter("wkvbv", [P, NKV, HLOC * DV], bf16, isOutput=False)
    wo_d = nc.declare_dram_parameter("wo", [P, HLOC, HID], f32r, isOutput=False)
    cosl_d = nc.declare_dram_parameter("cosl", [P, TC], f32, isOutput=False)
    sinl_d = nc.declare_dram_parameter("sinl", [P, TC], f32, isOutput=False)
    trimask_d = nc.declare_dram_parameter("trimask", [P, P], f32, isOutput=False)
    y_d = nc.declare_dram_parameter("y", [T, HID], f32, isOutput=True)

    # ---- dram bounce buffers for the collectives ----
    d_in = nc.dram_tensor("d_in", [P, 1], f32)
    d_out = nc.dram_tensor("d_out", [NC_, P, 1], f32, addr_space="Shared")
    # kv payload: 4 latent tiles + 1 roped kpe (dup) tile, [p, m, t]
    kv_in = nc.dram_tensor("kv_in", [P, NKV + 1, TC], bf16)
    kv_out = nc.dram_tensor(
        "kv_out", [NC_, P, NKV + 1, TC], bf16, addr_space="Shared"
    )
    # q payload: per dst head-pair [p, mo(3), t]: nope h0 | nope h1 | roped qpe
    q_in = nc.dram_tensor("q_in", [NP, P, 3, TC], bf16)
    q_out = nc.dram_tensor("q_out", [NC_, P, 3, TC], bf16)

    with tile.TileContext(nc) as tc, nc.allow_low_precision(
        reason="bf16/f32r matmul operands are intentional"
    ):
        # dummy collective: warms up the comm channel during stage-1
        nc.gpsimd.collective_compute(
            "AllGather",
            mybir.AluOpType.bypass,
            replica_groups=GRP,
            ins=[d_in[:, :].opt()],
            outs=[d_out[:, :, :].opt()],
        )

        with tc.tile_pool(name="persist", bufs=1) as pp:
            # persistent loads on the scalar queue (no waits -> no blocking)
            wkvbk_sb = pp.tile([P, NKV, HLOC * DN], bf16, name="wkvbk")
            nc.scalar.dma_start(out=wkvbk_sb, in_=wkvbk_d[:, :, :])
            wkvbv_sb = pp.tile([P, NKV, HLOC * DV], bf16, name="wkvbv")
            nc.scalar.dma_start(out=wkvbv_sb, in_=wkvbv_d[:, :, :])
            cosl_sb = pp.tile([P, TC], f32, name="cosl")
            nc.scalar.dma_start(out=cosl_sb, in_=cosl_d[:, :])
            sinl_sb = pp.tile([P, TC], f32, name="sinl")
            nc.scalar.dma_start(out=sinl_sb, in_=sinl_d[:, :])
            trimask_sb = pp.tile([P, P], f32, name="trimask")
            nc.scalar.dma_start(out=trimask_sb, in_=trimask_d[:, :])
            wo_sb = pp.tile([P, HLOC, HID], f32r, name="wo")
            nc.scalar.dma_start(out=wo_sb, in_=wo_d[:, :, :])

            ones_f = pp.tile([P, P], f32, name="ones_f")
            nc.vector.memset(ones_f, 1.0)
            ones_sb = pp.tile([P, 1], f32r, name="ones")
            nc.vector.tensor_copy(ones_sb, ones_f[:, 0:1])
            col_ones = pp.tile([1, P], f32r, name="col_ones")
            nc.vector.tensor_copy(col_ones, ones_f[0:1, :])
            zmask = pp.tile([P, HLOC], f32, name="zmask")
            nc.vector.memset(zmask[0:DR, 0:1], 1.0)
            nc.vector.memset(zmask[DR:P, 0:1], 0.0)
            nc.vector.memset(zmask[0:DR, 1:2], 0.0)
            nc.vector.memset(zmask[DR:P, 1:2], 1.0)
            eps_sb = pp.tile([1, 1], f32, name="eps")
            nc.vector.memset(eps_sb, EPS)

            pay_kv = pp.tile([P, NKV + 1, TC], bf16, name="paykv")
            # pay_q doubles as the post-AllToAll bf16 staging buffer
            pay_q = pp.tile([P, NP * 3, TC], bf16, name="payq")
            rq_b = pp.tile([P, TC], f32, name="rqb")
            rkv_b = pp.tile([P, TC], f32, name="rkvb")

            # attention matmul operands in f32r: measured bf16 moving
            # operands at free-dim 512 run 1.4x slower than f32r
            qTn = [pp.tile([P, T], f32r, name=f"qTn{h}") for h in range(HLOC)]
            qpeT2 = pp.tile([P, T], f32r, name="qpeT2")
            KT = [pp.tile([P, T], f32r, name=f"KT{h}") for h in range(HLOC)]
            kpe_raw = pp.tile([P, T], bf16, name="kperaw")
            kpe2 = [pp.tile([P, T], f32r, name=f"kpe2{h}") for h in range(HLOC)]
            kvn_sb = pp.tile([P, NKV, T], bf16, name="kvn")
            V_sb = [pp.tile([P, HLOC * DV], f32r, name=f"v{i}") for i in range(NKB)]

            # ---------------- Stage A: sharded projections ----------------
            with (
                tc.tile_pool(name="astream", bufs=3) as sp_,
                tc.tile_pool(name="aqbstream", bufs=3) as qbp,
                tc.tile_pool(name="asmall", bufs=1) as smp,
                tc.tile_pool(name="aps", bufs=3, space="PSUM") as s1ps,
                tc.tile_pool(name="arope", bufs=1, space="PSUM") as rps,
                tc.tile_pool(name="ssqps", bufs=1, space="PSUM") as ssqps,
                tc.tile_pool(name="upps", bufs=3, space="PSUM") as upps,
            ):
                h_sb = sp_.tile([P, NKH, TC], bf16, name="hchunk", bufs=1)
                nc.sync.dma_start(out=h_sb, in_=hT_d[:, :, :])
                qc_sb = sp_.tile([P, NKQ, TC], bf16, name="qc", bufs=1)
                ssq2 = ssqps.tile([1, 2 * TC], f32, name="ssq2")
                ssq_kv = ssq2[:, 0:TC]
                ssq_q = ssq2[:, TC : 2 * TC]

                # --- kv path first (its payload gates CC#1) ---
                rope_ps = []
                for m in range(NKV + 2):
                    wk_sb = sp_.tile([P, NKH, P], bf16, name="wstream")
                    nc.sync.dma_start(out=wk_sb, in_=wkva_d[:, m, :, :])
                    if m < NKV:
                        ps = s1ps.tile([P, TC], f32, name="s1")
                    else:
                        if m == NKV:
                            rope01 = rps.tile([P, 2 * TC], f32, name="rope01")
                        ps = rope01[:, (m - NKV) * TC : (m - NKV + 1) * TC]
                    for k in range(NKH):
                        nc.tensor.matmul(
                            ps,
                            lhsT=wk_sb[:, k, :],
                            rhs=h_sb[:, k, :],
                            start=(k == 0),
                            stop=(k == NKH - 1),
                        )
                    if m < NKV:
                        nc.vector.tensor_copy(pay_kv[:, m, :], ps)
                        sq = smp.tile([P, TC], f32r, name="sq", bufs=1)
                        nc.scalar.square(sq, ps)
                        nc.tensor.matmul(
                            ssq_kv,
                            lhsT=ones_sb,
                            rhs=sq,
                            start=(m == 0),
                            stop=(m == NKV - 1),
                        )
                    else:
                        rope_ps.append(ps)

                # rkv scale + broadcast
                rkv = smp.tile([1, TC], f32r, name="rkv")
                nc.scalar.activation(
                    rkv, ssq_kv, func=AF.Sqrt, bias=eps_sb, scale=1.0 / KVL
                )
                nc.vector.reciprocal(rkv, rkv)
                rkvb_ps = upps.tile([P, TC], f32, name="up")
                nc.tensor.matmul(rkvb_ps, lhsT=col_ones, rhs=rkv, start=True, stop=True)
                nc.vector.tensor_copy(rkv_b, rkvb_ps)
                # roped k_pe (dup rows), then normalize latent payload
                t1 = smp.tile([P, TC], f32, name="ropet1")
                t2 = smp.tile([P, TC], f32, name="ropet2")
                nc.vector.tensor_mul(t1, rope_ps[0], cosl_sb)
                nc.vector.tensor_mul(t2, rope_ps[1], sinl_sb)
                nc.vector.tensor_add(pay_kv[:, NKV, :], t1, t2)
                for m in range(NKV):
                    nc.vector.tensor_mul(pay_kv[:, m, :], pay_kv[:, m, :], rkv_b)
                nc.gpsimd.dma_start(out=kv_in[:, :, :], in_=pay_kv)
                nc.gpsimd.collective_compute(
                    "AllGather",
                    mybir.AluOpType.bypass,
                    replica_groups=GRP,
                    ins=[kv_in[:, :, :].opt()],
                    outs=[kv_out[:, :, :, :].opt()],
                )

                # --- q path stage-1 ---
                for m in range(NKQ):
                    wq_sb = sp_.tile([P, NKH, P], bf16, name="wstream")
                    nc.sync.dma_start(out=wq_sb, in_=wqa_d[:, m, :, :])
                    ps = s1ps.tile([P, TC], f32, name="s1")
                    for k in range(NKH):
                        nc.tensor.matmul(
                            ps,
                            lhsT=wq_sb[:, k, :],
                            rhs=h_sb[:, k, :],
                            start=(k == 0),
                            stop=(k == NKH - 1),
                        )
                    nc.vector.tensor_copy(qc_sb[:, m, :], ps)
                    sq = smp.tile([P, TC], f32r, name="sq", bufs=1)
                    nc.scalar.square(sq, ps)
                    nc.tensor.matmul(
                        ssq_q,
                        lhsT=ones_sb,
                        rhs=sq,
                        start=(m == 0),
                        stop=(m == NKQ - 1),
                    )
                rq = smp.tile([1, TC], f32r, name="rq")
                nc.scalar.activation(
                    rq, ssq_q, func=AF.Sqrt, bias=eps_sb, scale=1.0 / QL
                )
                nc.vector.reciprocal(rq, rq)
                rqb_ps = upps.tile([P, TC], f32, name="up")
                nc.tensor.matmul(rqb_ps, lhsT=col_ones, rhs=rq, start=True, stop=True)
                nc.vector.tensor_copy(rq_b, rqb_ps)

                # --- q up-projection: own tokens, ALL head-pairs ---
                for p_ in range(NP):
                    ups = []
                    for mo in range(4):
                        wqbs = qbp.tile([P, NKQ, P], bf16, name="wqbs")
                        nc.sync.dma_start(out=wqbs, in_=wqb_d[:, p_, mo, :, :])
                        ps = upps.tile([P, TC], f32, name="up")
                        for k in range(NKQ):
                            nc.tensor.matmul(
                                ps,
                                lhsT=wqbs[:, k, :],
                                rhs=qc_sb[:, k, :],
                                start=(k == 0),
                                stop=(k == NKQ - 1),
                            )
                        if mo < 2:
                            nc.vector.tensor_mul(
                                pay_q[:, p_ * 3 + mo, :], ps, rq_b
                            )
                        else:
                            ups.append(ps)
                    # neox rope on the decoupled dims, folded pre-exchange
                    t5 = smp.tile([P, TC], f32, name="qropet1")
                    t6 = smp.tile([P, TC], f32, name="qropet2")
                    nc.vector.tensor_mul(t5, ups[0], cosl_sb)
                    nc.vector.tensor_mul(t6, ups[1], sinl_sb)
                    nc.vector.tensor_add(t5, t5, t6)
                    nc.vector.tensor_mul(pay_q[:, p_ * 3 + 2, :], t5, rq_b)
                    nc.gpsimd.dma_start(
                        out=q_in[p_, :, :, :],
                        in_=pay_q[:, p_ * 3 : p_ * 3 + 3, :],
                    )
                nc.gpsimd.collective_compute(
                    "AllToAll",
                    mybir.AluOpType.bypass,
                    replica_groups=GRP,
                    ins=[q_in[:, :, :, :].opt()],
                    outs=[q_out[:, :, :, :].opt()],
                )

            # ---------------- Stage B: gather-side compute ----------------
            with (
                tc.tile_pool(name="bpt", bufs=6) as ptp,
                tc.tile_pool(name="bsmall", bufs=3) as bsm,
                tc.tile_pool(name="sps", bufs=4, space="PSUM") as spsp,
                tc.tile_pool(name="otps", bufs=2, space="PSUM") as otpsp,
                tc.tile_pool(name="lps", bufs=2, space="PSUM") as lpsp,
            ):
                # kv readback + K/V up-projection for own heads
                for r in range(NC_):
                    nc.scalar.dma_start(
                        out=kvn_sb[:, :, r * TC : (r + 1) * TC],
                        in_=kv_out[r, :, 0:NKV, :],
                    )
                    nc.scalar.dma_start(
                        out=kpe_raw[:, r * TC : (r + 1) * TC],
                        in_=kv_out[r, :, NKV, :],
                    )
                for h in range(HLOC):
                    nc.vector.tensor_scalar_mul(
                        kpe2[h], kpe_raw, zmask[:, h : h + 1]
                    )
                for h in range(HLOC):
                    for j in range(T // QC):
                        ps = otpsp.tile([P, QC], f32, name="otps")
                        for k in range(NKV):
                            nc.tensor.matmul(
                                ps,
                                lhsT=wkvbk_sb[:, k, h * P : (h + 1) * P],
                                rhs=kvn_sb[:, k, j * QC : (j + 1) * QC],
                                start=(k == 0),
                                stop=(k == NKV - 1),
                            )
                        nc.vector.tensor_copy(KT[h][:, j * QC : (j + 1) * QC], ps)
                for tt in range(NKB):
                    ps = otpsp.tile([P, QC], f32, name="otps")[:, : HLOC * DV]
                    for k in range(NKV):
                        nc.tensor.matmul(
                            ps,
                            lhsT=kvn_sb[:, k, tt * P : (tt + 1) * P],
                            rhs=wkvbv_sb[:, k, :],
                            start=(k == 0),
                            stop=(k == NKV - 1),
                        )
                    nc.vector.tensor_copy(V_sb[tt], ps)

                # q readback into pay_q (dead after the AllToAll consumed
                # q_in) then widen the moving operands to f32
                for r in range(NC_):
                    nc.scalar.dma_start(
                        out=pay_q[:, r, :], in_=q_out[r, :, 0, :]
                    )
                    nc.scalar.dma_start(
                        out=pay_q[:, NC_ + r, :], in_=q_out[r, :, 1, :]
                    )
                    nc.scalar.dma_start(
                        out=pay_q[:, 2 * NC_ + r, :], in_=q_out[r, :, 2, :]
                    )
                for h in range(HLOC):
                    nc.vector.tensor_copy(
                        qTn[h], pay_q[:, h * NC_ : (h + 1) * NC_, :]
                    )
                nc.vector.tensor_copy(qpeT2, pay_q[:, 2 * NC_ : 3 * NC_, :])

                # ---------------- attention ----------------
                OT_sb = [
                    [ptp.tile([P, QC], f32r, name=f"ot{h}_{j}", bufs=1) for j in range(NQC)]
                    for h in range(HLOC)
                ]

                def flush_norm(pend):
                    p_ot, p_l, p_h, p_j = pend
                    recl = bsm.tile([1, QC], f32r, name="recl")
                    nc.vector.reciprocal(recl, p_l)
                    lb_ps = spsp.tile([P, QC], f32, name="sps2")
                    nc.tensor.matmul(lb_ps, lhsT=col_ones, rhs=recl, start=True, stop=True)
                    lb = bsm.tile([P, QC], f32, name="lb")
                    nc.scalar.copy(lb, lb_ps)
                    nc.vector.tensor_mul(OT_sb[p_h][p_j], p_ot, lb)

                pend = None
                for h in range(HLOC):
                    for j in range(NQC):
                        ot_ps = otpsp.tile([P, QC], f32, name="otps")
                        l_ps = lpsp.tile([1, QC], f32, name="lps")
                        nkb = 4 * (j + 1)
                        qcol0 = j * QC
                        for ki in range(nkb):
                            s2 = spsp.tile([P, QC], f32, name="sps2")
                            nc.tensor.matmul(
                                s2,
                                lhsT=KT[h][:, ki * P : (ki + 1) * P],
                                rhs=qTn[h][:, qcol0 : qcol0 + QC],
                                start=True,
                                stop=False,
                            )
                            nc.tensor.matmul(
                                s2,
                                lhsT=kpe2[h][:, ki * P : (ki + 1) * P],
                                rhs=qpeT2[:, qcol0 : qcol0 + QC],
                                start=False,
                                stop=True,
                            )
                            pt = ptp.tile([P, QC], f32r, name="pt")
                            nc.scalar.activation(pt, s2, func=AF.Exp, scale=SCALE)
                            diag = (ki // 4 == j)
                            cs = (ki % 4) * P if diag else 0
                            if diag:
                                nc.gpsimd.tensor_mul(
                                    pt[:, cs : cs + P],
                                    pt[:, cs : cs + P],
                                    trimask_sb,
                                )
                            nc.tensor.matmul(
                                ot_ps[:, cs:],
                                lhsT=V_sb[ki][:, h * DV : (h + 1) * DV],
                                rhs=pt[:, cs:],
                                start=(ki == 0),
                                stop=(ki == nkb - 1),
                            )
                            nc.tensor.matmul(
                                l_ps[:, cs:],
                                lhsT=ones_sb,
                                rhs=pt[:, cs:],
                                start=(ki == 0),
                                stop=(ki == nkb - 1),
                            )
                            if ki == 2 and pend is not None:
                                flush_norm(pend)
                                pend = None
                        pend = (ot_ps, l_ps, h, j)
                flush_norm(pend)

                # ---------------- o_proj ----------------
                for tt in range(T // P):
                    j, sub = tt // 4, (tt % 4) * P
                    for n in range(HID // QC):
                        y_ps = spsp.tile([P, QC], f32, name="sps2")
                        for h in range(HLOC):
                            nc.tensor.matmul(
                                y_ps,
                                lhsT=OT_sb[h][j][:, sub : sub + P],
                                rhs=wo_sb[:, h, n * QC : (n + 1) * QC],
                                start=(h == 0),
                                stop=(h == HLOC - 1),
                            )
                        y_sb = ptp.tile([P, QC], f32, name="ysb")
                        nc.vector.tensor_copy(y_sb, y_ps)
                        nc.sync.dma_start(
                            out=y_d[tt * P : (tt + 1) * P, n * QC : (n + 1) * QC],
                            in_=y_sb,
                        )
    _split_multi_waits(nc, mybir)
    return nc


def _host_prep(inputs):
    import ml_dtypes

    bf = ml_dtypes.bfloat16
    hs = np.ascontiguousarray(np.asarray(inputs["hidden_states"], np.float32))
    pos = np.asarray(inputs["positions"], np.int32)
    w_qa = np.asarray(inputs["w_qa"], np.float32)
    q_ln = np.asarray(inputs["q_a_ln_w"], np.float32)
    w_qb = np.asarray(inputs["w_qb"], np.float32)
    w_kva = np.asarray(inputs["w_kva"], np.float32)
    kv_ln = np.asarray(inputs["kv_a_ln_w"], np.float32)
    w_kvb = np.asarray(inputs["w_kvb"], np.float32)
    w_o = np.asarray(inputs["w_o"], np.float32)

    # a-projections, pre-tiled: [p, m, k, col]
    wqa_b = np.ascontiguousarray(
        w_qa.reshape(NKH, P, NKQ, P).transpose(1, 2, 0, 3)
    ).astype(bf)

    def rot_cols(A):
        return np.concatenate([-A[:, DR // 2 :], A[:, : DR // 2]], axis=1)

    kva_lat = w_kva[:, :KVL]
    kva_rope = w_kva[:, KVL:]                      # [2048, 64]
    kva_ropeB = rot_cols(kva_rope)
    wkva_aug = np.concatenate(
        [kva_lat, kva_rope, kva_rope, kva_ropeB, kva_ropeB], axis=1
    )  # [2048, 512+128+128]
    wkva_b = np.ascontiguousarray(
        wkva_aug.reshape(NKH, P, NKV + 2, P).transpose(1, 2, 0, 3)
    ).astype(bf)

    # rope tables (dup-row structure)
    inv_freq = (
        1.0 / (10000.0 ** (np.arange(0, DR, 2, dtype=np.float32) / DR))
    ).astype(np.float32)
    freqs = pos.astype(np.float32)[:, None] * inv_freq[None, :]
    emb = np.concatenate([freqs, freqs], axis=-1)  # [T, 64]
    cosT = np.ascontiguousarray(np.cos(emb).T.astype(np.float32))  # [64, T]
    sinT = np.ascontiguousarray(np.sin(emb).T.astype(np.float32))
    cos2 = np.ascontiguousarray(np.concatenate([cosT, cosT], axis=0))  # [128, T]
    sin2 = np.ascontiguousarray(np.concatenate([sinT, sinT], axis=0))

    # q b-projection, ALL head-pairs, ln folded: [p, pair, mo, k, col]
    w_qb_f = (w_qb * q_ln[:, None]).reshape(QL, H, DQK)
    blocks = []
    for p_ in range(NP):
        h0, h1 = 2 * p_, 2 * p_ + 1
        ropeA = np.concatenate(
            [w_qb_f[:, h0, DN:], w_qb_f[:, h1, DN:]], axis=1
        )  # [QL, 128]
        ropeB = np.concatenate(
            [rot_cols(w_qb_f[:, h0, DN:]), rot_cols(w_qb_f[:, h1, DN:])], axis=1
        )
        blocks.append(
            np.stack(
                [w_qb_f[:, h0, :DN], w_qb_f[:, h1, :DN], ropeA, ropeB], axis=0
            )  # [4, QL, 128]
        )
    wqb_all = np.stack(blocks, axis=0)  # [NP, 4, QL, 128]
    wqb_aug = np.ascontiguousarray(
        wqb_all.reshape(NP, 4, NKQ, P, P).transpose(3, 0, 1, 2, 4)
    ).astype(bf)  # [p, pair, mo, k, col]

    w_kvb_f = (w_kvb * kv_ln[:, None]).reshape(KVL, H, DN + DV)
    w_o_r = w_o.reshape(H, DV, HID)
    trimask = np.triu(np.ones((P, P), dtype=np.float32))

    per_core = []
    for i in range(NC_):
        hh = [HLOC * i + x for x in range(HLOC)]
        t0 = i * TC
        hT = np.ascontiguousarray(
            hs[t0 : t0 + TC].reshape(TC, NKH, P).transpose(2, 1, 0)
        ).astype(bf)
        wkvbk = np.ascontiguousarray(
            np.concatenate([w_kvb_f[:, h, :DN] for h in hh], axis=1)
            .reshape(NKV, P, HLOC * DN)
            .transpose(1, 0, 2)
        ).astype(bf)
        wkvbv = np.ascontiguousarray(
            np.concatenate([w_kvb_f[:, h, DN:] for h in hh], axis=1)
            .reshape(NKV, P, HLOC * DV)
            .transpose(1, 0, 2)
        ).astype(bf)
        wo_i = np.ascontiguousarray(
            np.stack([w_o_r[h] for h in hh], axis=0).transpose(1, 0, 2)
        )  # [p, h, HID] f32
        per_core.append(
            dict(
                hT=hT,
                wqa=wqa_b,
                wkva=wkva_b,
                wqb=wqb_aug,
                wkvbk=wkvbk,
                wkvbv=wkvbv,
                wo=wo_i,
                cosl=np.ascontiguousarray(cos2[:, t0 : t0 + TC]),
                sinl=np.ascontiguousarray(sin2[:, t0 : t0 + TC]),
                trimask=trimask,
            )
        )
    return per_core


def kernel(**inputs):
    global LAST_RESULTS
    from concourse.bass_utils import run_bass_kernel_spmd

    if "nc" not in _CACHE:
        _CACHE["nc"] = _build_program()
    nc = _CACHE["nc"]

    in_maps = _host_prep(inputs)
    res = run_bass_kernel_spmd(nc, in_maps, core_ids=list(range(NC_)))
    LAST_RESULTS = res
    out = np.zeros((T, HID), dtype=np.float32)
    for r in res.results:
        out += np.asarray(r["y"], dtype=np.float32)
    return out
